# revision 16
# baseline (speedup 1.0000x reference)
"""Trainium2 Bass kernel for a custom LSTM cell.

Math (per reference):
    i = sigmoid(x @ W_i.T + b_Wi + h @ U_i.T + b_Ui)
    f = sigmoid(x @ W_f.T + b_Wf + h @ U_f.T + b_Uf + boundary @ W_b.T + b_Wb)
    o = sigmoid(x @ W_o.T + b_Wo + h @ U_o.T + b_Uo)
    g = tanh   (x @ W_g.T + b_Wg + h @ U_g.T + b_Ug)
    c = f * c_prev + i * g
    h = o * tanh(c)

Strategy: data-parallel over batch across 8 NeuronCores (1024 rows each).
Unlike the batch-on-partitions baseline, PSUM output tiles put GATE dims on
partitions and batch on the free axis (out = M_tile.T @ A_tile).  That lets
the per-gate bias ride the scalar-engine activation instruction (per-partition
bias + scale), removing all dedicated bias matmuls from the PE.

Operands are bf16 (same PE rate as f32r, half the LDWEIGHTS and DMA cost).
The last NFP8 of the 8 h-contraction subtiles run as fp8e4m3 DoubleRow
matmuls (2 K-subtiles per instruction, 2x PE throughput).  All matmul
operands on the weight side are pre-scaled by 128 on the host so the fp8
U-weights sit in e4m3's normal range; the activation instruction's
scale=1/128 undoes it before sigmoid/tanh.

Host marshalling pre-arranges every tensor into the exact SBUF layout
(partition-major), so all DMAs are contiguous per partition.
"""

import sys

sys.path.insert(0, "/opt/trn_rl_repo")

import numpy as np
import ml_dtypes

B, IN, H = 8192, 512, 1024
NCORES = 8
BLOC = B // NCORES  # 1024 batch rows per core
KTOT = IN + H  # 1536 contraction
KT = KTOT // 128  # 12 K-subtiles total
NFP8 = 6  # of the 8 h-side K-subtiles, how many run fp8 DoubleRow (even)
KBF = KT - NFP8  # leading bf16 K-subtiles (x part + first h subtiles)
NG = 4 * H // 128  # 32 gate-tiles of 128 output dims
NS = H // 128  # 8 h-slices
NCH = BLOC // 512  # 2 batch chunks of 512
SCALE = 128.0  # weight pre-scale (power of two; undone in activation)

_PROG = None  # cached so repeat calls skip rebuild/recompile


def _build_program():
    import concourse.bass as bass
    import concourse.mybir as mybir
    import concourse.tile as tile
    from concourse import bacc
    from contextlib import ExitStack

    f32 = mybir.dt.float32
    bf16 = mybir.dt.bfloat16
    f8 = mybir.dt.float8e4
    SIG = mybir.ActivationFunctionType.Sigmoid
    TANH = mybir.ActivationFunctionType.Tanh
    DR = mybir.MatmulPerfMode.DoubleRow

    nc = bacc.Bacc("TRN2", target_bir_lowering=False, debug=False)

    # Device tensors, all in final SBUF layout (partition dim first).
    # m_bf: [128, NG, KBF, 128] bf16   weights, K-subtiles 0..KBF-1
    # m_f8: [128, NG, NFP8, 128] fp8   weights, K-subtiles KBF..11
    # at_bf: [128, KBF, BLOC] bf16     activations [x | h_lo]
    # at_f8: [128, NFP8, BLOC] fp8     activations h_hi
    # bias: [128, NG] f32              per gate-tile per partition
    # bd:   [2, BLOC] bf16             boundary.T
    # wb:   [2, H] bf16                W_b.T * SCALE
    # c_in: [128, NS, BLOC] f32        c_prev.T
    # h/c out: [128, NS, BLOC] f32     transposed outputs
    m_bf_d = nc.dram_tensor("m_bf", [128, NG * KBF * 128], bf16, kind="ExternalInput").ap()
    at_bf_d = nc.dram_tensor("at_bf", [128, KBF * BLOC], bf16, kind="ExternalInput").ap()
    bias_d = nc.dram_tensor("bias_in", [128, NG], f32, kind="ExternalInput").ap()
    bd_d = nc.dram_tensor("bd_in", [2, BLOC], bf16, kind="ExternalInput").ap()
    wb_d = nc.dram_tensor("wb_in", [2, H], bf16, kind="ExternalInput").ap()
    c_d = nc.dram_tensor("c_in", [128, NS * BLOC], bf16, kind="ExternalInput").ap()
    # merged output: slot (ch, s) holds [c' | h'] as [128, 1024]
    hc_o = nc.dram_tensor(
        "hc_out", [128, NCH * NS * 1024], bf16, kind="ExternalOutput"
    ).ap()
    if NFP8:
        m_f8_d = nc.dram_tensor(
            "m_f8", [128, NG * NFP8 * 128], f8, kind="ExternalInput"
        ).ap()
        at_f8_d = nc.dram_tensor(
            "at_f8", [128, NFP8 * BLOC], f8, kind="ExternalInput"
        ).ap()

    with tile.TileContext(nc) as tc:
        with ExitStack() as ctx:
            wup = ctx.enter_context(tc.tile_pool(name="wup", bufs=1))
            cst = ctx.enter_context(tc.tile_pool(name="cst", bufs=1))
            wp = ctx.enter_context(tc.tile_pool(name="wp", bufs=1))
            actp = ctx.enter_context(tc.tile_pool(name="actp", bufs=2))
            outp = ctx.enter_context(tc.tile_pool(name="outp", bufs=4))
            psp = ctx.enter_context(tc.tile_pool(name="psp", bufs=8, space="PSUM"))

            # PE warm-up: dummy bf16 matmuls with no DMA deps push the PE
            # through its p-state ramp while the first weights load.
            wu_w = wup.tile([128, 128], bf16, name="wu_w")
            nc.vector.memset(wu_w, 0.0)
            wu_ps = psp.tile([128, 512], f32, name="wu_ps", tag="ps")
            for _ in range(48):
                nc.tensor.matmul(wu_ps[:, 0:128], wu_w, wu_w, start=True, stop=True)

            # Queue plan (DMA engines are shared; sem pool is small, so few,
            # fat DMAs, with the first-needed data leading each queue):
            #  - sync/Q1:  activations, one DMA per K-subtile (PE start gate)
            #  - scalar/Q10: tiny constants, then weights as one fat DMA per
            #    4-gate-tile group in compute order (big per-partition runs
            #    -> big packets -> ~250GB/s)
            #  - gpsimd/Q0: c_prev (bf16, 4 DMAs), then output stores
            at_bf_t = cst.tile([128, KBF, BLOC], bf16, name="at_bf_t")
            at_f8_t = cst.tile([128, NFP8, BLOC], f8, name="at_f8_t") if NFP8 else None
            for k in range(KBF):
                nc.sync.dma_start(
                    out=at_bf_t[:, k], in_=at_bf_d[:, k * BLOC : (k + 1) * BLOC]
                )
            for kp in range(0, NFP8, 2):
                nc.sync.dma_start(
                    out=at_f8_t[:, kp : kp + 2],
                    in_=at_f8_d[:, kp * BLOC : (kp + 2) * BLOC].rearrange(
                        "p (k b) -> p k b", k=2
                    ),
                )

            bias_t = cst.tile([128, NG], f32, name="bias_t")
            nc.scalar.dma_start(out=bias_t, in_=bias_d[:, :])
            bd_t = cst.tile([2, BLOC], bf16, name="bd_t")
            nc.scalar.dma_start(out=bd_t, in_=bd_d[:, :])
            wb_t = cst.tile([2, H], bf16, name="wb_t")
            nc.scalar.dma_start(out=wb_t, in_=wb_d[:, :])

            # weights: host already permuted gate-tiles into compute order
            # (gorder); gi = position in that order
            gorder = [s + 8 * z for s in range(NS) for z in range(4)]
            w_bf = {}
            w_f8 = {}
            for gi in range(0, NG, 4):
                t = wp.tile([128, 4, KBF, 128], bf16, name=f"wbf_g{gi}")
                nc.scalar.dma_start(
                    out=t,
                    in_=m_bf_d[
                        :, gi * KBF * 128 : (gi + 4) * KBF * 128
                    ].rearrange("p (j k c) -> p j k c", j=4, k=KBF),
                )
                t8 = None
                if NFP8:
                    t8 = wp.tile([128, 4, NFP8, 128], f8, name=f"wf8_g{gi}")
                    nc.scalar.dma_start(
                        out=t8,
                        in_=m_f8_d[
                            :, gi * NFP8 * 128 : (gi + 4) * NFP8 * 128
                        ].rearrange("p (j k c) -> p j k c", j=4, k=NFP8),
                    )
                for j in range(4):
                    g = gorder[gi + j]
                    w_bf[g] = t[:, j]
                    if NFP8:
                        w_f8[g] = t8[:, j]

            c_t = cst.tile([128, NS, BLOC], bf16, name="c_t")
            for sp in range(0, NS, 2):
                nc.gpsimd.dma_start(
                    out=c_t[:, sp : sp + 2],
                    in_=c_d[:, sp * BLOC : (sp + 2) * BLOC].rearrange(
                        "p (s b) -> p s b", s=2
                    ),
                )

            for ch in range(NCH):
                cs = slice(ch * 512, (ch + 1) * 512)
                for s in range(NS):
                    # gate order i, f, o, g -> gate-tile ids s, 8+s, 16+s, 24+s
                    ps = {}
                    for z in range(4):
                        g = 8 * z + s
                        p = psp.tile([128, 512], f32, name=f"ps{ch}_{s}_{z}", tag="ps")
                        ps[z] = p
                        if z == 1:
                            # boundary influence seeds the f-gate accumulator
                            nc.tensor.matmul(
                                p, wb_t[:, s * 128 : (s + 1) * 128], bd_t[:, cs],
                                start=True, stop=False,
                            )
                        for k in range(KBF):
                            nc.tensor.matmul(
                                p, w_bf[g][:, k, :], at_bf_t[:, k, cs],
                                start=(k == 0 and z != 1),
                                stop=(NFP8 == 0 and k == KBF - 1),
                            )
                        for kp in range(0, NFP8, 2):
                            nc.tensor.matmul(
                                p,
                                w_f8[g][:, kp : kp + 2, :],
                                at_f8_t[:, kp : kp + 2, cs],
                                start=False,
                                stop=(kp == NFP8 - 2),
                                perf_mode=DR,
                            )

                    # activations: sigmoid(i,f,o), tanh(g); bias+scale folded in
                    gt = {}
                    for z, fn in ((0, SIG), (1, SIG), (2, SIG), (3, TANH)):
                        g = 8 * z + s
                        t = actp.tile([128, 512], bf16, name=f"g{ch}_{s}_{z}", tag=f"g{z}")
                        nc.scalar.activation(
                            t, ps[z], fn, bias=bias_t[:, g : g + 1], scale=1.0 / SCALE
                        )
                        gt[z] = t

                    # c' = f*c + i*g ; h = o*tanh(c')  (bf16 elementwise: 2x DVE)
                    hc = outp.tile([128, 1024], bf16, name=f"hc{ch}_{s}", tag="hc")
                    cn = hc[:, 0:512]
                    hn = hc[:, 512:1024]
                    tmp = actp.tile([128, 512], bf16, name=f"tmp{ch}_{s}", tag="tmp")
                    nc.vector.tensor_mul(cn, gt[1], c_t[:, s, cs])
                    nc.vector.tensor_mul(tmp, gt[0], gt[3])
                    nc.vector.tensor_add(cn, cn, tmp)
                    th = actp.tile([128, 512], bf16, name=f"th{ch}_{s}", tag="th")
                    nc.scalar.activation(th, cn, TANH)
                    nc.vector.tensor_mul(hn, gt[2], th)

                    off = (ch * NS + s) * 1024
                    nc.gpsimd.dma_start(out=hc_o[:, off : off + 1024], in_=hc)
    nc.compile()
    return nc


def _get_program():
    global _PROG
    if _PROG is None:
        _PROG = _build_program()
    return _PROG


def _prep_inputs(inputs):
    """Host-side marshalling into exact SBUF layouts (see header)."""
    f = np.float32
    bf = ml_dtypes.bfloat16
    f8 = ml_dtypes.float8_e4m3
    x = np.asarray(inputs["x"], f)
    h_prev = np.asarray(inputs["h_prev"], f)
    c_prev = np.asarray(inputs["c_prev"], f)
    boundary = np.asarray(inputs["boundary"], f)

    gates = ["i", "f", "o", "g"]
    # M [1536, 4096]: rows = contraction (x then h), cols = [i|f|o|g] x H.
    M = np.empty((KTOT, 4 * H), f)
    bias_vec = np.empty(4 * H, f)
    for zi, z in enumerate(gates):
        W = np.asarray(inputs[f"W_{z}"], f)
        U = np.asarray(inputs[f"U_{z}"], f)
        cs = slice(zi * H, (zi + 1) * H)
        M[:IN, cs] = W.T
        M[IN:, cs] = U.T
        b = np.asarray(inputs[f"b_W{z}"], f) + np.asarray(inputs[f"b_U{z}"], f)
        if z == "f":
            b = b + np.asarray(inputs["b_Wb"], f)
        bias_vec[cs] = b
    M *= SCALE

    # [K, 4H] -> [128, NG, KS, 128]: K-row = 128*kk + p, col = 128*g + c,
    # with gate-tiles permuted into compute order (s-major, then gate)
    gorder = [s + 8 * z for s in range(NS) for z in range(4)]

    def dev_weights(Msub, ks):
        t = Msub.reshape(ks, 128, NG, 128).transpose(1, 2, 0, 3)[:, gorder]
        return np.ascontiguousarray(t).reshape(128, -1)

    m_bf = dev_weights(M[: KBF * 128], KBF).astype(bf)
    bias_dev = np.ascontiguousarray(bias_vec.reshape(NG, 128).T)  # [128, NG]
    wb_dev = np.ascontiguousarray(
        (np.asarray(inputs["W_b"], f).T * SCALE).astype(bf)
    )  # [2, H]

    AT = np.concatenate([x, h_prev], axis=1).T  # [1536, 8192] (full batch)
    at_bf_full = np.ascontiguousarray(
        AT[: KBF * 128].reshape(KBF, 128, B).transpose(1, 0, 2)
    ).astype(bf)  # [128, KBF, B]
    cT = c_prev.T  # [H, B]

    in_maps = []
    if NFP8:
        m_f8 = dev_weights(M[KBF * 128 :], NFP8).astype(f8)
        at_f8_full = np.ascontiguousarray(
            AT[KBF * 128 :].reshape(NFP8, 128, B).transpose(1, 0, 2)
        ).astype(f8)  # [128, NFP8, B]
    for c in range(NCORES):
        rs = slice(c * BLOC, (c + 1) * BLOC)
        im = {
            "m_bf": m_bf,
            "at_bf": np.ascontiguousarray(at_bf_full[:, :, rs]).reshape(128, -1),
            "bias_in": bias_dev,
            "bd_in": np.ascontiguousarray(boundary[rs].T.astype(bf)),
            "wb_in": wb_dev,
            "c_in": np.ascontiguousarray(
                cT[:, rs].reshape(NS, 128, BLOC).transpose(1, 0, 2)
            ).reshape(128, -1).astype(bf),
        }
        if NFP8:
            im["m_f8"] = m_f8
            im["at_f8"] = np.ascontiguousarray(at_f8_full[:, :, rs]).reshape(128, -1)
        in_maps.append(im)
    return in_maps


def _unshard(res_list):
    """hc_out [128, NCH*NS*1024] bf16 per core -> (h, c) [B, H] f32 full."""
    h_parts, c_parts = [], []
    for r in res_list:
        t = np.asarray(r["hc_out"], dtype=np.float32)
        # [128, ch, s, half(c|h), 512] -> [h=128*s, b=512*ch+...]
        t = t.reshape(128, NCH, NS, 2, 512)
        core_c = t[:, :, :, 0, :].transpose(2, 0, 1, 3).reshape(H, BLOC)
        core_h = t[:, :, :, 1, :].transpose(2, 0, 1, 3).reshape(H, BLOC)
        c_parts.append(core_c.T)
        h_parts.append(core_h.T)
    return (
        np.ascontiguousarray(np.concatenate(h_parts, axis=0)),
        np.ascontiguousarray(np.concatenate(c_parts, axis=0)),
    )


def run(inputs, trace=False):
    """Returns ((h, c), BassKernelResults)."""
    from concourse.bass_utils import run_bass_kernel_spmd

    nc = _get_program()
    in_maps = _prep_inputs(inputs)
    res = run_bass_kernel_spmd(
        nc, in_maps, core_ids=list(range(NCORES)), trace=trace
    )
    h, c = _unshard(res.results)
    return (h, c), res


def kernel(**inputs):
    out, _ = run(inputs, trace=False)
    return out


# revision 19
# speedup vs baseline: 1.0543x; 1.0543x over previous
"""Trainium2 Bass kernel for a custom LSTM cell.

Math (per reference):
    i = sigmoid(x @ W_i.T + b_Wi + h @ U_i.T + b_Ui)
    f = sigmoid(x @ W_f.T + b_Wf + h @ U_f.T + b_Uf + boundary @ W_b.T + b_Wb)
    o = sigmoid(x @ W_o.T + b_Wo + h @ U_o.T + b_Uo)
    g = tanh   (x @ W_g.T + b_Wg + h @ U_g.T + b_Ug)
    c = f * c_prev + i * g
    h = o * tanh(c)

Strategy: data-parallel over batch across 8 NeuronCores (1024 rows each).
Unlike the batch-on-partitions baseline, PSUM output tiles put GATE dims on
partitions and batch on the free axis (out = M_tile.T @ A_tile).  That lets
the per-gate bias ride the scalar-engine activation instruction (per-partition
bias + scale), removing all dedicated bias matmuls from the PE.

Operands are bf16 (same PE rate as f32r, half the LDWEIGHTS and DMA cost).
The last NFP8 of the 8 h-contraction subtiles run as fp8e4m3 DoubleRow
matmuls (2 K-subtiles per instruction, 2x PE throughput).  All matmul
operands on the weight side are pre-scaled by 128 on the host so the fp8
U-weights sit in e4m3's normal range; the activation instruction's
scale=1/128 undoes it before sigmoid/tanh.

Host marshalling pre-arranges every tensor into the exact SBUF layout
(partition-major), so all DMAs are contiguous per partition.
"""

import sys

sys.path.insert(0, "/opt/trn_rl_repo")

import numpy as np
import ml_dtypes

B, IN, H = 8192, 512, 1024
NCORES = 8
BLOC = B // NCORES  # 1024 batch rows per core
KTOT = IN + H  # 1536 contraction
KT = KTOT // 128  # 12 K-subtiles total
NFP8 = 6  # of the 8 h-side K-subtiles, how many run fp8 DoubleRow (even)
KBF = KT - NFP8  # leading bf16 K-subtiles (x part + first h subtiles)
NG = 4 * H // 128  # 32 gate-tiles of 128 output dims
NS = H // 128  # 8 h-slices
NCH = BLOC // 512  # 2 batch chunks of 512
SCALE = 128.0  # weight pre-scale (power of two; undone in activation)

_PROG = None  # cached so repeat calls skip rebuild/recompile


def _build_program():
    import concourse.bass as bass
    import concourse.mybir as mybir
    import concourse.tile as tile
    from concourse import bacc
    from contextlib import ExitStack

    f32 = mybir.dt.float32
    bf16 = mybir.dt.bfloat16
    f8 = mybir.dt.float8e4
    SIG = mybir.ActivationFunctionType.Sigmoid
    TANH = mybir.ActivationFunctionType.Tanh
    DR = mybir.MatmulPerfMode.DoubleRow

    nc = bacc.Bacc("TRN2", target_bir_lowering=False, debug=False)

    # Device tensors, all in final SBUF layout (partition dim first).
    # m_bf: [128, NG, KBF, 128] bf16   weights, K-subtiles 0..KBF-1
    # m_f8: [128, NG, NFP8, 128] fp8   weights, K-subtiles KBF..11
    # at_bf: [128, KBF, BLOC] bf16     activations [x | h_lo]
    # at_f8: [128, NFP8, BLOC] fp8     activations h_hi
    # bias: [128, NG] f32              per gate-tile per partition
    # bd:   [2, BLOC] bf16             boundary.T
    # wb:   [2, H] bf16                W_b.T * SCALE
    # c_in: [128, NS, BLOC] f32        c_prev.T
    # h/c out: [128, NS, BLOC] f32     transposed outputs
    m_bf_d = nc.dram_tensor("m_bf", [128, NG * KBF * 128], bf16, kind="ExternalInput").ap()
    at_bf_d = nc.dram_tensor("at_bf", [128, KBF * BLOC], bf16, kind="ExternalInput").ap()
    bias_d = nc.dram_tensor("bias_in", [128, NG], f32, kind="ExternalInput").ap()
    bd_d = nc.dram_tensor("bd_in", [2, BLOC], bf16, kind="ExternalInput").ap()
    wb_d = nc.dram_tensor("wb_in", [2, H], bf16, kind="ExternalInput").ap()
    c_d = nc.dram_tensor("c_in", [128, NS * BLOC], bf16, kind="ExternalInput").ap()
    # merged output: slot (ch, s) holds [c' | h'] as [128, 1024]
    hc_o = nc.dram_tensor(
        "hc_out", [128, NCH * NS * 1024], bf16, kind="ExternalOutput"
    ).ap()
    if NFP8:
        m_f8_d = nc.dram_tensor(
            "m_f8", [128, NG * NFP8 * 128], f8, kind="ExternalInput"
        ).ap()
        at_f8_d = nc.dram_tensor(
            "at_f8", [128, NFP8 * BLOC], f8, kind="ExternalInput"
        ).ap()

    with tile.TileContext(nc) as tc:
        with ExitStack() as ctx:
            wup = ctx.enter_context(tc.tile_pool(name="wup", bufs=1))
            cst = ctx.enter_context(tc.tile_pool(name="cst", bufs=1))
            wp = ctx.enter_context(tc.tile_pool(name="wp", bufs=1))
            actp = ctx.enter_context(tc.tile_pool(name="actp", bufs=2))
            outp = ctx.enter_context(tc.tile_pool(name="outp", bufs=4))
            psp = ctx.enter_context(tc.tile_pool(name="psp", bufs=8, space="PSUM"))

            # PE warm-up: dummy bf16 matmuls with no DMA deps push the PE
            # through its p-state ramp while the first weights load.
            wu_w = wup.tile([128, 128], bf16, name="wu_w")
            nc.vector.memset(wu_w, 0.0)
            wu_ps = psp.tile([128, 512], f32, name="wu_ps", tag="ps")
            for _ in range(48):
                nc.tensor.matmul(wu_ps[:, 0:128], wu_w, wu_w, start=True, stop=True)

            # Queue plan (DMA engines are shared; sem pool is small, so few,
            # fat DMAs, with the first-needed data leading each queue):
            #  - sync/Q1:  activations, one DMA per K-subtile (PE start gate)
            #  - scalar/Q10: tiny constants, then weights as one fat DMA per
            #    4-gate-tile group in compute order (big per-partition runs
            #    -> big packets -> ~250GB/s)
            #  - gpsimd/Q0: c_prev (bf16, 4 DMAs), then output stores
            at_bf_t = cst.tile([128, KBF, BLOC], bf16, name="at_bf_t")
            at_f8_t = cst.tile([128, NFP8, BLOC], f8, name="at_f8_t") if NFP8 else None
            for k in range(KBF):
                nc.sync.dma_start(
                    out=at_bf_t[:, k], in_=at_bf_d[:, k * BLOC : (k + 1) * BLOC]
                )
            for kp in range(0, NFP8, 2):
                nc.sync.dma_start(
                    out=at_f8_t[:, kp : kp + 2],
                    in_=at_f8_d[:, kp * BLOC : (kp + 2) * BLOC].rearrange(
                        "p (k b) -> p k b", k=2
                    ),
                )

            bias_t = cst.tile([128, NG], f32, name="bias_t")
            nc.scalar.dma_start(out=bias_t, in_=bias_d[:, :])
            bd_t = cst.tile([2, BLOC], bf16, name="bd_t")
            nc.scalar.dma_start(out=bd_t, in_=bd_d[:, :])
            wb_t = cst.tile([2, H], bf16, name="wb_t")
            nc.scalar.dma_start(out=wb_t, in_=wb_d[:, :])

            # weights: host already permuted gate-tiles into compute order
            # (gorder); weight group j = the 4 gates of h-slice j.  Groups 0-1
            # load up front; later groups are issued from inside the compute
            # loop so the in-order scalar queue paces them behind compute,
            # keeping early DMA bandwidth for the activations.
            gorder = [s + 8 * z for s in range(NS) for z in range(4)]
            w_bf = {}
            w_f8 = {}
            wgrp_tiles = []
            for gi in range(0, NG, 4):
                t = wp.tile([128, 4, KBF, 128], bf16, name=f"wbf_g{gi}")
                t8 = (
                    wp.tile([128, 4, NFP8, 128], f8, name=f"wf8_g{gi}")
                    if NFP8
                    else None
                )
                wgrp_tiles.append((t, t8))
                for j in range(4):
                    g = gorder[gi + j]
                    w_bf[g] = t[:, j]
                    if NFP8:
                        w_f8[g] = t8[:, j]

            def load_wgrp(grp):
                gi = grp * 4
                t, t8 = wgrp_tiles[grp]
                nc.scalar.dma_start(
                    out=t,
                    in_=m_bf_d[
                        :, gi * KBF * 128 : (gi + 4) * KBF * 128
                    ].rearrange("p (j k c) -> p j k c", j=4, k=KBF),
                )
                if NFP8:
                    nc.scalar.dma_start(
                        out=t8,
                        in_=m_f8_d[
                            :, gi * NFP8 * 128 : (gi + 4) * NFP8 * 128
                        ].rearrange("p (j k c) -> p j k c", j=4, k=NFP8),
                    )

            load_wgrp(0)
            load_wgrp(1)

            # c_prev after the activations on the sync queue (needed ~2 gates
            # into each h-slice, so mild lateness is benign)
            c_t = cst.tile([128, NS, BLOC], bf16, name="c_t")
            for sp in range(0, NS, 2):
                nc.sync.dma_start(
                    out=c_t[:, sp : sp + 2],
                    in_=c_d[:, sp * BLOC : (sp + 2) * BLOC].rearrange(
                        "p (s b) -> p s b", s=2
                    ),
                )

            # z order i, g, f, o: i*g computes mid-iteration, f*c + tanh(c')
            # during o's matmuls, so the post-last-psum chain is just o's
            # activation, h = o*th, and the store.
            SEQ = (0, 3, 1, 2)
            FN = {0: SIG, 1: SIG, 2: SIG, 3: TANH}
            for ch in range(NCH):
                cs = slice(ch * 512, (ch + 1) * 512)
                for s in range(NS):
                    if ch == 0 and s < NS - 2:
                        load_wgrp(s + 2)
                    # gate z -> gate-tile id 8*z + s (weights in gorder tiles)
                    ps = {}
                    gt = {}
                    hc = outp.tile([128, 1024], bf16, name=f"hc{ch}_{s}", tag="hc")
                    cn = hc[:, 0:512]
                    hn = hc[:, 512:1024]
                    tmp = actp.tile([128, 512], bf16, name=f"tmp{ch}_{s}", tag="tmp")
                    th = actp.tile([128, 512], bf16, name=f"th{ch}_{s}", tag="th")
                    for zi, z in enumerate(SEQ):
                        g = 8 * z + s
                        p = psp.tile([128, 512], f32, name=f"ps{ch}_{s}_{z}", tag="ps")
                        ps[z] = p
                        if z == 1:
                            # boundary influence seeds the f-gate accumulator
                            nc.tensor.matmul(
                                p, wb_t[:, s * 128 : (s + 1) * 128], bd_t[:, cs],
                                start=True, stop=False,
                            )
                        for k in range(KBF):
                            nc.tensor.matmul(
                                p, w_bf[g][:, k, :], at_bf_t[:, k, cs],
                                start=(k == 0 and z != 1),
                                stop=(NFP8 == 0 and k == KBF - 1),
                            )
                        for kp in range(0, NFP8, 2):
                            nc.tensor.matmul(
                                p,
                                w_f8[g][:, kp : kp + 2, :],
                                at_f8_t[:, kp : kp + 2, cs],
                                start=False,
                                stop=(kp == NFP8 - 2),
                                perf_mode=DR,
                            )
                        t = actp.tile(
                            [128, 512], bf16, name=f"g{ch}_{s}_{z}", tag=f"g{z}"
                        )
                        nc.scalar.activation(
                            t, p, FN[z], bias=bias_t[:, g : g + 1], scale=1.0 / SCALE
                        )
                        gt[z] = t
                        # interleave elementwise as operands become ready
                        if zi == 1:  # i, g done
                            nc.vector.tensor_mul(tmp, gt[0], gt[3])
                        elif zi == 2:  # f done
                            nc.vector.tensor_mul(cn, gt[1], c_t[:, s, cs])
                            nc.vector.tensor_add(cn, cn, tmp)
                            nc.scalar.activation(th, cn, TANH)
                    nc.vector.tensor_mul(hn, gt[2], th)

                    off = (ch * NS + s) * 1024
                    nc.scalar.dma_start(out=hc_o[:, off : off + 1024], in_=hc)
    nc.compile()
    return nc


def _get_program():
    global _PROG
    if _PROG is None:
        _PROG = _build_program()
    return _PROG


def _prep_inputs(inputs):
    """Host-side marshalling into exact SBUF layouts (see header)."""
    f = np.float32
    bf = ml_dtypes.bfloat16
    f8 = ml_dtypes.float8_e4m3
    x = np.asarray(inputs["x"], f)
    h_prev = np.asarray(inputs["h_prev"], f)
    c_prev = np.asarray(inputs["c_prev"], f)
    boundary = np.asarray(inputs["boundary"], f)

    gates = ["i", "f", "o", "g"]
    # M [1536, 4096]: rows = contraction (x then h), cols = [i|f|o|g] x H.
    M = np.empty((KTOT, 4 * H), f)
    bias_vec = np.empty(4 * H, f)
    for zi, z in enumerate(gates):
        W = np.asarray(inputs[f"W_{z}"], f)
        U = np.asarray(inputs[f"U_{z}"], f)
        cs = slice(zi * H, (zi + 1) * H)
        M[:IN, cs] = W.T
        M[IN:, cs] = U.T
        b = np.asarray(inputs[f"b_W{z}"], f) + np.asarray(inputs[f"b_U{z}"], f)
        if z == "f":
            b = b + np.asarray(inputs["b_Wb"], f)
        bias_vec[cs] = b
    M *= SCALE

    # [K, 4H] -> [128, NG, KS, 128]: K-row = 128*kk + p, col = 128*g + c,
    # with gate-tiles permuted into compute order (s-major, then gate)
    gorder = [s + 8 * z for s in range(NS) for z in range(4)]

    def dev_weights(Msub, ks):
        t = Msub.reshape(ks, 128, NG, 128).transpose(1, 2, 0, 3)[:, gorder]
        return np.ascontiguousarray(t).reshape(128, -1)

    m_bf = dev_weights(M[: KBF * 128], KBF).astype(bf)
    bias_dev = np.ascontiguousarray(bias_vec.reshape(NG, 128).T)  # [128, NG]
    wb_dev = np.ascontiguousarray(
        (np.asarray(inputs["W_b"], f).T * SCALE).astype(bf)
    )  # [2, H]

    AT = np.concatenate([x, h_prev], axis=1).T  # [1536, 8192] (full batch)
    at_bf_full = np.ascontiguousarray(
        AT[: KBF * 128].reshape(KBF, 128, B).transpose(1, 0, 2)
    ).astype(bf)  # [128, KBF, B]
    cT = c_prev.T  # [H, B]

    in_maps = []
    if NFP8:
        m_f8 = dev_weights(M[KBF * 128 :], NFP8).astype(f8)
        at_f8_full = np.ascontiguousarray(
            AT[KBF * 128 :].reshape(NFP8, 128, B).transpose(1, 0, 2)
        ).astype(f8)  # [128, NFP8, B]
    for c in range(NCORES):
        rs = slice(c * BLOC, (c + 1) * BLOC)
        im = {
            "m_bf": m_bf,
            "at_bf": np.ascontiguousarray(at_bf_full[:, :, rs]).reshape(128, -1),
            "bias_in": bias_dev,
            "bd_in": np.ascontiguousarray(boundary[rs].T.astype(bf)),
            "wb_in": wb_dev,
            "c_in": np.ascontiguousarray(
                cT[:, rs].reshape(NS, 128, BLOC).transpose(1, 0, 2)
            ).reshape(128, -1).astype(bf),
        }
        if NFP8:
            im["m_f8"] = m_f8
            im["at_f8"] = np.ascontiguousarray(at_f8_full[:, :, rs]).reshape(128, -1)
        in_maps.append(im)
    return in_maps


def _unshard(res_list):
    """hc_out [128, NCH*NS*1024] bf16 per core -> (h, c) [B, H] f32 full."""
    h_parts, c_parts = [], []
    for r in res_list:
        t = np.asarray(r["hc_out"], dtype=np.float32)
        # [128, ch, s, half(c|h), 512] -> [h=128*s, b=512*ch+...]
        t = t.reshape(128, NCH, NS, 2, 512)
        core_c = t[:, :, :, 0, :].transpose(2, 0, 1, 3).reshape(H, BLOC)
        core_h = t[:, :, :, 1, :].transpose(2, 0, 1, 3).reshape(H, BLOC)
        c_parts.append(core_c.T)
        h_parts.append(core_h.T)
    return (
        np.ascontiguousarray(np.concatenate(h_parts, axis=0)),
        np.ascontiguousarray(np.concatenate(c_parts, axis=0)),
    )


def run(inputs, trace=False):
    """Returns ((h, c), BassKernelResults)."""
    from concourse.bass_utils import run_bass_kernel_spmd

    nc = _get_program()
    in_maps = _prep_inputs(inputs)
    res = run_bass_kernel_spmd(
        nc, in_maps, core_ids=list(range(NCORES)), trace=trace
    )
    h, c = _unshard(res.results)
    return (h, c), res


def kernel(**inputs):
    out, _ = run(inputs, trace=False)
    return out


# revision 22
# speedup vs baseline: 1.1528x; 1.0934x over previous
"""Trainium2 Bass kernel for a custom LSTM cell.

Math (per reference):
    i = sigmoid(x @ W_i.T + b_Wi + h @ U_i.T + b_Ui)
    f = sigmoid(x @ W_f.T + b_Wf + h @ U_f.T + b_Uf + boundary @ W_b.T + b_Wb)
    o = sigmoid(x @ W_o.T + b_Wo + h @ U_o.T + b_Uo)
    g = tanh   (x @ W_g.T + b_Wg + h @ U_g.T + b_Ug)
    c = f * c_prev + i * g
    h = o * tanh(c)

Strategy: data-parallel over batch across 8 NeuronCores (1024 rows each).
Unlike the batch-on-partitions baseline, PSUM output tiles put GATE dims on
partitions and batch on the free axis (out = M_tile.T @ A_tile).  That lets
the per-gate bias ride the scalar-engine activation instruction (per-partition
bias + scale), removing all dedicated bias matmuls from the PE.

Operands are bf16 (same PE rate as f32r, half the LDWEIGHTS and DMA cost).
The last NFP8 of the 8 h-contraction subtiles run as fp8e4m3 DoubleRow
matmuls (2 K-subtiles per instruction, 2x PE throughput).  All matmul
operands on the weight side are pre-scaled by 128 on the host so the fp8
U-weights sit in e4m3's normal range; the activation instruction's
scale=1/128 undoes it before sigmoid/tanh.

Host marshalling pre-arranges every tensor into the exact SBUF layout
(partition-major), so all DMAs are contiguous per partition.
"""

import sys

sys.path.insert(0, "/opt/trn_rl_repo")

import numpy as np
import ml_dtypes

B, IN, H = 8192, 512, 1024
NCORES = 8
BLOC = B // NCORES  # 1024 batch rows per core
KTOT = IN + H  # 1536 contraction
KT = KTOT // 128  # 12 K-subtiles total
NFP8 = 8  # of the 8 h-side K-subtiles, how many run fp8 DoubleRow (even)
KBF = KT - NFP8  # leading bf16 K-subtiles (x part + first h subtiles)
NG = 4 * H // 128  # 32 gate-tiles of 128 output dims
NS = H // 128  # 8 h-slices
NCH = BLOC // 512  # 2 batch chunks of 512
SCALE = 128.0  # weight pre-scale (power of two; undone in activation)

_PROG = None  # cached so repeat calls skip rebuild/recompile


def _build_program():
    import concourse.bass as bass
    import concourse.mybir as mybir
    import concourse.tile as tile
    from concourse import bacc
    from contextlib import ExitStack

    f32 = mybir.dt.float32
    bf16 = mybir.dt.bfloat16
    f8 = mybir.dt.float8e4
    SIG = mybir.ActivationFunctionType.Sigmoid
    TANH = mybir.ActivationFunctionType.Tanh
    DR = mybir.MatmulPerfMode.DoubleRow

    nc = bacc.Bacc("TRN2", target_bir_lowering=False, debug=False)

    # Device tensors, all in final SBUF layout (partition dim first).
    # m_bf: [128, NG, KBF, 128] bf16   weights, K-subtiles 0..KBF-1
    # m_f8: [128, NG, NFP8, 128] fp8   weights, K-subtiles KBF..11
    # at_bf: [128, KBF, BLOC] bf16     activations [x | h_lo]
    # at_f8: [128, NFP8, BLOC] fp8     activations h_hi
    # bias: [128, NG] f32              per gate-tile per partition
    # bd:   [2, BLOC] bf16             boundary.T
    # wb:   [2, H] bf16                W_b.T * SCALE
    # c_in: [128, NS, BLOC] f32        c_prev.T
    # h/c out: [128, NS, BLOC] f32     transposed outputs
    m_bf_d = nc.dram_tensor("m_bf", [128, NG * KBF * 128], bf16, kind="ExternalInput").ap()
    at_bf_d = nc.dram_tensor("at_bf", [128, KBF * BLOC], bf16, kind="ExternalInput").ap()
    bias_d = nc.dram_tensor("bias_in", [128, NG], f32, kind="ExternalInput").ap()
    bd_d = nc.dram_tensor("bd_in", [2, BLOC], bf16, kind="ExternalInput").ap()
    wb_d = nc.dram_tensor("wb_in", [2, H], bf16, kind="ExternalInput").ap()
    c_d = nc.dram_tensor("c_in", [128, NS * BLOC], bf16, kind="ExternalInput").ap()
    # merged output: slot (ch, s) holds [c' | h'] as [128, 1024]
    hc_o = nc.dram_tensor(
        "hc_out", [128, NCH * NS * 1024], bf16, kind="ExternalOutput"
    ).ap()
    if NFP8:
        m_f8_d = nc.dram_tensor(
            "m_f8", [128, NG * NFP8 * 128], f8, kind="ExternalInput"
        ).ap()
        at_f8_d = nc.dram_tensor(
            "at_f8", [128, NFP8 * BLOC], f8, kind="ExternalInput"
        ).ap()

    with tile.TileContext(nc) as tc:
        with ExitStack() as ctx:
            wup = ctx.enter_context(tc.tile_pool(name="wup", bufs=1))
            cst = ctx.enter_context(tc.tile_pool(name="cst", bufs=1))
            wp = ctx.enter_context(tc.tile_pool(name="wp", bufs=1))
            actp = ctx.enter_context(tc.tile_pool(name="actp", bufs=2))
            outp = ctx.enter_context(tc.tile_pool(name="outp", bufs=4))
            psp = ctx.enter_context(tc.tile_pool(name="psp", bufs=8, space="PSUM"))

            # PE warm-up: dummy bf16 matmuls with no DMA deps push the PE
            # through its p-state ramp while the first weights load.
            wu_w = wup.tile([128, 128], bf16, name="wu_w")
            nc.vector.memset(wu_w, 0.0)
            wu_ps = psp.tile([128, 512], f32, name="wu_ps", tag="ps")
            for _ in range(48):
                nc.tensor.matmul(wu_ps[:, 0:128], wu_w, wu_w, start=True, stop=True)

            # Queue plan (DMA engines are shared; sem pool is small, so few,
            # fat DMAs, with the first-needed data leading each queue):
            #  - sync/Q1:  activations, one DMA per K-subtile (PE start gate)
            #  - scalar/Q10: tiny constants, then weights as one fat DMA per
            #    4-gate-tile group in compute order (big per-partition runs
            #    -> big packets -> ~250GB/s)
            #  - gpsimd/Q0: c_prev (bf16, 4 DMAs), then output stores
            at_bf_t = cst.tile([128, KBF, BLOC], bf16, name="at_bf_t")
            at_f8_t = cst.tile([128, NFP8, BLOC], f8, name="at_f8_t") if NFP8 else None
            for k in range(KBF):
                nc.sync.dma_start(
                    out=at_bf_t[:, k], in_=at_bf_d[:, k * BLOC : (k + 1) * BLOC]
                )
            for kp in range(0, NFP8, 2):
                nc.gpsimd.dma_start(
                    out=at_f8_t[:, kp : kp + 2],
                    in_=at_f8_d[:, kp * BLOC : (kp + 2) * BLOC].rearrange(
                        "p (k b) -> p k b", k=2
                    ),
                )

            bias_t = cst.tile([128, NG], f32, name="bias_t")
            nc.scalar.dma_start(out=bias_t, in_=bias_d[:, :])
            bd_t = cst.tile([2, BLOC], bf16, name="bd_t")
            nc.scalar.dma_start(out=bd_t, in_=bd_d[:, :])
            wb_t = cst.tile([2, H], bf16, name="wb_t")
            nc.scalar.dma_start(out=wb_t, in_=wb_d[:, :])

            # weights: host already permuted gate-tiles into compute order
            # (gorder); weight group j = the 4 gates of h-slice j.  Groups 0-1
            # load up front; later groups are issued from inside the compute
            # loop so the in-order scalar queue paces them behind compute,
            # keeping early DMA bandwidth for the activations.
            gorder = [s + 8 * z for s in range(NS) for z in range(4)]
            w_bf = {}
            w_f8 = {}
            wgrp_tiles = []
            for gi in range(0, NG, 4):
                t = wp.tile([128, 4, KBF, 128], bf16, name=f"wbf_g{gi}")
                t8 = (
                    wp.tile([128, 4, NFP8, 128], f8, name=f"wf8_g{gi}")
                    if NFP8
                    else None
                )
                wgrp_tiles.append((t, t8))
                for j in range(4):
                    g = gorder[gi + j]
                    w_bf[g] = t[:, j]
                    if NFP8:
                        w_f8[g] = t8[:, j]

            def load_wgrp(grp, split=False):
                gi = grp * 4
                t, t8 = wgrp_tiles[grp]
                if split:  # per-gate-tile DMAs so the first tile lands early
                    for j in range(4):
                        nc.scalar.dma_start(
                            out=t[:, j],
                            in_=m_bf_d[
                                :, (gi + j) * KBF * 128 : (gi + j + 1) * KBF * 128
                            ].rearrange("p (k c) -> p k c", k=KBF),
                        )
                        if NFP8:
                            nc.scalar.dma_start(
                                out=t8[:, j],
                                in_=m_f8_d[
                                    :,
                                    (gi + j) * NFP8 * 128 : (gi + j + 1) * NFP8 * 128,
                                ].rearrange("p (k c) -> p k c", k=NFP8),
                            )
                    return
                nc.scalar.dma_start(
                    out=t,
                    in_=m_bf_d[
                        :, gi * KBF * 128 : (gi + 4) * KBF * 128
                    ].rearrange("p (j k c) -> p j k c", j=4, k=KBF),
                )
                if NFP8:
                    nc.scalar.dma_start(
                        out=t8,
                        in_=m_f8_d[
                            :, gi * NFP8 * 128 : (gi + 4) * NFP8 * 128
                        ].rearrange("p (j k c) -> p j k c", j=4, k=NFP8),
                    )

            load_wgrp(0, split=True)
            load_wgrp(1)

            # c_prev after the activations on the sync queue (needed ~2 gates
            # into each h-slice, so mild lateness is benign)
            c_t = cst.tile([128, NS, BLOC], bf16, name="c_t")
            for sp in range(0, NS, 2):
                nc.sync.dma_start(
                    out=c_t[:, sp : sp + 2],
                    in_=c_d[:, sp * BLOC : (sp + 2) * BLOC].rearrange(
                        "p (s b) -> p s b", s=2
                    ),
                )

            # z order i, g, f, o: i*g computes mid-iteration, f*c + tanh(c')
            # during o's matmuls, so the post-last-psum chain is just o's
            # activation, h = o*th, and the store.
            SEQ = (0, 3, 1, 2)
            FN = {0: SIG, 1: SIG, 2: SIG, 3: TANH}
            for ch in range(NCH):
                cs = slice(ch * 512, (ch + 1) * 512)
                for s in range(NS):
                    if ch == 0 and s < NS - 2:
                        load_wgrp(s + 2)
                    # gate z -> gate-tile id 8*z + s (weights in gorder tiles)
                    ps = {}
                    gt = {}
                    hc = outp.tile([128, 1024], bf16, name=f"hc{ch}_{s}", tag="hc")
                    cn = hc[:, 0:512]
                    hn = hc[:, 512:1024]
                    tmp = actp.tile([128, 512], bf16, name=f"tmp{ch}_{s}", tag="tmp")
                    th = actp.tile([128, 512], bf16, name=f"th{ch}_{s}", tag="th")
                    for zi, z in enumerate(SEQ):
                        g = 8 * z + s
                        p = psp.tile([128, 512], f32, name=f"ps{ch}_{s}_{z}", tag="ps")
                        ps[z] = p
                        if z == 1:
                            # boundary influence seeds the f-gate accumulator
                            nc.tensor.matmul(
                                p, wb_t[:, s * 128 : (s + 1) * 128], bd_t[:, cs],
                                start=True, stop=False,
                            )
                        for k in range(KBF):
                            nc.tensor.matmul(
                                p, w_bf[g][:, k, :], at_bf_t[:, k, cs],
                                start=(k == 0 and z != 1),
                                stop=(NFP8 == 0 and k == KBF - 1),
                            )
                        for kp in range(0, NFP8, 2):
                            nc.tensor.matmul(
                                p,
                                w_f8[g][:, kp : kp + 2, :],
                                at_f8_t[:, kp : kp + 2, cs],
                                start=False,
                                stop=(kp == NFP8 - 2),
                                perf_mode=DR,
                            )
                        t = actp.tile(
                            [128, 512], bf16, name=f"g{ch}_{s}_{z}", tag=f"g{z}"
                        )
                        nc.scalar.activation(
                            t, p, FN[z], bias=bias_t[:, g : g + 1], scale=1.0 / SCALE
                        )
                        gt[z] = t
                        # interleave elementwise as operands become ready
                        if zi == 1:  # i, g done
                            nc.vector.tensor_mul(tmp, gt[0], gt[3])
                        elif zi == 2:  # f done
                            nc.vector.tensor_mul(cn, gt[1], c_t[:, s, cs])
                            nc.vector.tensor_add(cn, cn, tmp)
                            nc.scalar.activation(th, cn, TANH)
                    nc.vector.tensor_mul(hn, gt[2], th)

                    off = (ch * NS + s) * 1024
                    nc.scalar.dma_start(out=hc_o[:, off : off + 1024], in_=hc)
    nc.compile()
    return nc


def _get_program():
    global _PROG
    if _PROG is None:
        _PROG = _build_program()
    return _PROG


def _prep_inputs(inputs):
    """Host-side marshalling into exact SBUF layouts (see header)."""
    f = np.float32
    bf = ml_dtypes.bfloat16
    f8 = ml_dtypes.float8_e4m3
    x = np.asarray(inputs["x"], f)
    h_prev = np.asarray(inputs["h_prev"], f)
    c_prev = np.asarray(inputs["c_prev"], f)
    boundary = np.asarray(inputs["boundary"], f)

    gates = ["i", "f", "o", "g"]
    # M [1536, 4096]: rows = contraction (x then h), cols = [i|f|o|g] x H.
    M = np.empty((KTOT, 4 * H), f)
    bias_vec = np.empty(4 * H, f)
    for zi, z in enumerate(gates):
        W = np.asarray(inputs[f"W_{z}"], f)
        U = np.asarray(inputs[f"U_{z}"], f)
        cs = slice(zi * H, (zi + 1) * H)
        M[:IN, cs] = W.T
        M[IN:, cs] = U.T
        b = np.asarray(inputs[f"b_W{z}"], f) + np.asarray(inputs[f"b_U{z}"], f)
        if z == "f":
            b = b + np.asarray(inputs["b_Wb"], f)
        bias_vec[cs] = b
    M *= SCALE

    # [K, 4H] -> [128, NG, KS, 128]: K-row = 128*kk + p, col = 128*g + c,
    # with gate-tiles permuted into compute order (s-major, then gate)
    gorder = [s + 8 * z for s in range(NS) for z in range(4)]

    def dev_weights(Msub, ks):
        t = Msub.reshape(ks, 128, NG, 128).transpose(1, 2, 0, 3)[:, gorder]
        return np.ascontiguousarray(t).reshape(128, -1)

    m_bf = dev_weights(M[: KBF * 128], KBF).astype(bf)
    bias_dev = np.ascontiguousarray(bias_vec.reshape(NG, 128).T)  # [128, NG]
    wb_dev = np.ascontiguousarray(
        (np.asarray(inputs["W_b"], f).T * SCALE).astype(bf)
    )  # [2, H]

    AT = np.concatenate([x, h_prev], axis=1).T  # [1536, 8192] (full batch)
    at_bf_full = np.ascontiguousarray(
        AT[: KBF * 128].reshape(KBF, 128, B).transpose(1, 0, 2)
    ).astype(bf)  # [128, KBF, B]
    cT = c_prev.T  # [H, B]

    in_maps = []
    if NFP8:
        m_f8 = dev_weights(M[KBF * 128 :], NFP8).astype(f8)
        at_f8_full = np.ascontiguousarray(
            AT[KBF * 128 :].reshape(NFP8, 128, B).transpose(1, 0, 2)
        ).astype(f8)  # [128, NFP8, B]
    for c in range(NCORES):
        rs = slice(c * BLOC, (c + 1) * BLOC)
        im = {
            "m_bf": m_bf,
            "at_bf": np.ascontiguousarray(at_bf_full[:, :, rs]).reshape(128, -1),
            "bias_in": bias_dev,
            "bd_in": np.ascontiguousarray(boundary[rs].T.astype(bf)),
            "wb_in": wb_dev,
            "c_in": np.ascontiguousarray(
                cT[:, rs].reshape(NS, 128, BLOC).transpose(1, 0, 2)
            ).reshape(128, -1).astype(bf),
        }
        if NFP8:
            im["m_f8"] = m_f8
            im["at_f8"] = np.ascontiguousarray(at_f8_full[:, :, rs]).reshape(128, -1)
        in_maps.append(im)
    return in_maps


def _unshard(res_list):
    """hc_out [128, NCH*NS*1024] bf16 per core -> (h, c) [B, H] f32 full."""
    h_parts, c_parts = [], []
    for r in res_list:
        t = np.asarray(r["hc_out"], dtype=np.float32)
        # [128, ch, s, half(c|h), 512] -> [h=128*s, b=512*ch+...]
        t = t.reshape(128, NCH, NS, 2, 512)
        core_c = t[:, :, :, 0, :].transpose(2, 0, 1, 3).reshape(H, BLOC)
        core_h = t[:, :, :, 1, :].transpose(2, 0, 1, 3).reshape(H, BLOC)
        c_parts.append(core_c.T)
        h_parts.append(core_h.T)
    return (
        np.ascontiguousarray(np.concatenate(h_parts, axis=0)),
        np.ascontiguousarray(np.concatenate(c_parts, axis=0)),
    )


def run(inputs, trace=False):
    """Returns ((h, c), BassKernelResults)."""
    from concourse.bass_utils import run_bass_kernel_spmd

    nc = _get_program()
    in_maps = _prep_inputs(inputs)
    res = run_bass_kernel_spmd(
        nc, in_maps, core_ids=list(range(NCORES)), trace=trace
    )
    h, c = _unshard(res.results)
    return (h, c), res


def kernel(**inputs):
    out, _ = run(inputs, trace=False)
    return out


# revision 24
# speedup vs baseline: 1.1581x; 1.0046x over previous
"""Trainium2 Bass kernel for a custom LSTM cell.

Math (per reference):
    i = sigmoid(x @ W_i.T + b_Wi + h @ U_i.T + b_Ui)
    f = sigmoid(x @ W_f.T + b_Wf + h @ U_f.T + b_Uf + boundary @ W_b.T + b_Wb)
    o = sigmoid(x @ W_o.T + b_Wo + h @ U_o.T + b_Uo)
    g = tanh   (x @ W_g.T + b_Wg + h @ U_g.T + b_Ug)
    c = f * c_prev + i * g
    h = o * tanh(c)

Strategy: data-parallel over batch across 8 NeuronCores (1024 rows each).
Unlike the batch-on-partitions baseline, PSUM output tiles put GATE dims on
partitions and batch on the free axis (out = M_tile.T @ A_tile).  That lets
the per-gate bias ride the scalar-engine activation instruction (per-partition
bias + scale), removing all dedicated bias matmuls from the PE.

Operands are bf16 (same PE rate as f32r, half the LDWEIGHTS and DMA cost).
The last NFP8 of the 8 h-contraction subtiles run as fp8e4m3 DoubleRow
matmuls (2 K-subtiles per instruction, 2x PE throughput).  All matmul
operands on the weight side are pre-scaled by 128 on the host so the fp8
U-weights sit in e4m3's normal range; the activation instruction's
scale=1/128 undoes it before sigmoid/tanh.

Host marshalling pre-arranges every tensor into the exact SBUF layout
(partition-major), so all DMAs are contiguous per partition.
"""

import sys

sys.path.insert(0, "/opt/trn_rl_repo")

import numpy as np
import ml_dtypes

B, IN, H = 8192, 512, 1024
NCORES = 8
BLOC = B // NCORES  # 1024 batch rows per core
KTOT = IN + H  # 1536 contraction
KT = KTOT // 128  # 12 K-subtiles total
NFP8 = 8  # of the 8 h-side K-subtiles, how many run fp8 DoubleRow (even)
KBF = KT - NFP8  # leading bf16 K-subtiles (x part + first h subtiles)
NG = 4 * H // 128  # 32 gate-tiles of 128 output dims
NS = H // 128  # 8 h-slices
NCH = BLOC // 512  # 2 batch chunks of 512
SCALE = 128.0  # weight pre-scale (power of two; undone in activation)

_PROG = None  # cached so repeat calls skip rebuild/recompile


def _build_program():
    import concourse.bass as bass
    import concourse.mybir as mybir
    import concourse.tile as tile
    from concourse import bacc
    from contextlib import ExitStack

    f32 = mybir.dt.float32
    bf16 = mybir.dt.bfloat16
    f8 = mybir.dt.float8e4
    SIG = mybir.ActivationFunctionType.Sigmoid
    TANH = mybir.ActivationFunctionType.Tanh
    DR = mybir.MatmulPerfMode.DoubleRow

    nc = bacc.Bacc("TRN2", target_bir_lowering=False, debug=False)

    # Device tensors, all in final SBUF layout (partition dim first).
    # m_bf: [128, NG, KBF, 128] bf16   weights, K-subtiles 0..KBF-1
    # m_f8: [128, NG, NFP8, 128] fp8   weights, K-subtiles KBF..11
    # at_bf: [128, KBF, BLOC] bf16     activations [x | h_lo]
    # at_f8: [128, NFP8, BLOC] fp8     activations h_hi
    # bias: [128, NG] f32              per gate-tile per partition
    # bd:   [2, BLOC] bf16             boundary.T
    # wb:   [2, H] bf16                W_b.T * SCALE
    # c_in: [128, NS, BLOC] f32        c_prev.T
    # h/c out: [128, NS, BLOC] f32     transposed outputs
    m_bf_d = nc.dram_tensor("m_bf", [128, NG * KBF * 128], bf16, kind="ExternalInput").ap()
    at_bf_d = nc.dram_tensor("at_bf", [128, KBF * BLOC], bf16, kind="ExternalInput").ap()
    bias_d = nc.dram_tensor("bias_in", [128, NG], f32, kind="ExternalInput").ap()
    bd_d = nc.dram_tensor("bd_in", [2, BLOC], bf16, kind="ExternalInput").ap()
    wb_d = nc.dram_tensor("wb_in", [2, H], bf16, kind="ExternalInput").ap()
    c_d = nc.dram_tensor("c_in", [128, NS * BLOC], bf16, kind="ExternalInput").ap()
    # merged output: slot (ch, s) holds [c' | h'] as [128, 1024]
    hc_o = nc.dram_tensor(
        "hc_out", [128, NCH * NS * 1024], bf16, kind="ExternalOutput"
    ).ap()
    if NFP8:
        m_f8_d = nc.dram_tensor(
            "m_f8", [128, NG * NFP8 * 128], f8, kind="ExternalInput"
        ).ap()
        at_f8_d = nc.dram_tensor(
            "at_f8", [128, NFP8 * BLOC], f8, kind="ExternalInput"
        ).ap()

    with tile.TileContext(nc) as tc:
        with ExitStack() as ctx:
            wup = ctx.enter_context(tc.tile_pool(name="wup", bufs=1))
            cst = ctx.enter_context(tc.tile_pool(name="cst", bufs=1))
            wp = ctx.enter_context(tc.tile_pool(name="wp", bufs=1))
            actp = ctx.enter_context(tc.tile_pool(name="actp", bufs=2))
            outp = ctx.enter_context(tc.tile_pool(name="outp", bufs=4))
            psp = ctx.enter_context(tc.tile_pool(name="psp", bufs=8, space="PSUM"))

            # PE warm-up: dummy bf16 matmuls with no DMA deps push the PE
            # through its p-state ramp while the first weights load.
            wu_w = wup.tile([128, 128], bf16, name="wu_w")
            nc.vector.memset(wu_w, 0.0)
            wu_ps = psp.tile([128, 512], f32, name="wu_ps", tag="ps")
            for _ in range(48):
                nc.tensor.matmul(wu_ps[:, 0:128], wu_w, wu_w, start=True, stop=True)

            # Queue plan (DMA engines are shared; sem pool is small, so few,
            # fat DMAs, with the first-needed data leading each queue):
            #  - sync/Q1:  activations, one DMA per K-subtile (PE start gate)
            #  - scalar/Q10: tiny constants, then weights as one fat DMA per
            #    4-gate-tile group in compute order (big per-partition runs
            #    -> big packets -> ~250GB/s)
            #  - gpsimd/Q0: c_prev (bf16, 4 DMAs), then output stores
            at_bf_t = cst.tile([128, KBF, BLOC], bf16, name="at_bf_t")
            at_f8_t = cst.tile([128, NFP8, BLOC], f8, name="at_f8_t") if NFP8 else None
            for k in range(KBF):
                nc.sync.dma_start(
                    out=at_bf_t[:, k], in_=at_bf_d[:, k * BLOC : (k + 1) * BLOC]
                )
            for kp in range(0, NFP8, 2):
                nc.gpsimd.dma_start(
                    out=at_f8_t[:, kp : kp + 2],
                    in_=at_f8_d[:, kp * BLOC : (kp + 2) * BLOC].rearrange(
                        "p (k b) -> p k b", k=2
                    ),
                )

            bias_t = cst.tile([128, NG], f32, name="bias_t")
            nc.scalar.dma_start(out=bias_t, in_=bias_d[:, :])
            bd_t = cst.tile([2, BLOC], bf16, name="bd_t")
            nc.scalar.dma_start(out=bd_t, in_=bd_d[:, :])
            wb_t = cst.tile([2, H], bf16, name="wb_t")
            nc.scalar.dma_start(out=wb_t, in_=wb_d[:, :])

            # weights: host already permuted gate-tiles into compute order
            # (gorder); weight group j = the 4 gates of h-slice j.  Groups 0-1
            # load up front; later groups are issued from inside the compute
            # loop so the in-order scalar queue paces them behind compute,
            # keeping early DMA bandwidth for the activations.
            gorder = [s + 8 * z for s in range(NS) for z in range(4)]
            w_bf = {}
            w_f8 = {}
            wgrp_tiles = []
            for gi in range(0, NG, 4):
                t = wp.tile([128, 4, KBF, 128], bf16, name=f"wbf_g{gi}")
                t8 = (
                    wp.tile([128, 4, NFP8, 128], f8, name=f"wf8_g{gi}")
                    if NFP8
                    else None
                )
                wgrp_tiles.append((t, t8))
                for j in range(4):
                    g = gorder[gi + j]
                    w_bf[g] = t[:, j]
                    if NFP8:
                        w_f8[g] = t8[:, j]

            def load_wgrp(grp, split=False):
                gi = grp * 4
                t, t8 = wgrp_tiles[grp]
                if split:  # per-gate-tile DMAs so the first tile lands early
                    for j in range(4):
                        nc.scalar.dma_start(
                            out=t[:, j],
                            in_=m_bf_d[
                                :, (gi + j) * KBF * 128 : (gi + j + 1) * KBF * 128
                            ].rearrange("p (k c) -> p k c", k=KBF),
                        )
                        if NFP8:
                            nc.scalar.dma_start(
                                out=t8[:, j],
                                in_=m_f8_d[
                                    :,
                                    (gi + j) * NFP8 * 128 : (gi + j + 1) * NFP8 * 128,
                                ].rearrange("p (k c) -> p k c", k=NFP8),
                            )
                    return
                nc.scalar.dma_start(
                    out=t,
                    in_=m_bf_d[
                        :, gi * KBF * 128 : (gi + 4) * KBF * 128
                    ].rearrange("p (j k c) -> p j k c", j=4, k=KBF),
                )
                if NFP8:
                    nc.scalar.dma_start(
                        out=t8,
                        in_=m_f8_d[
                            :, gi * NFP8 * 128 : (gi + 4) * NFP8 * 128
                        ].rearrange("p (j k c) -> p j k c", j=4, k=NFP8),
                    )

            load_wgrp(0, split=True)
            load_wgrp(1)

            # c_prev after the activations on the sync queue (needed ~2 gates
            # into each h-slice, so mild lateness is benign)
            c_t = cst.tile([128, NS, BLOC], bf16, name="c_t")
            for sp in range(0, NS, 2):
                nc.sync.dma_start(
                    out=c_t[:, sp : sp + 2],
                    in_=c_d[:, sp * BLOC : (sp + 2) * BLOC].rearrange(
                        "p (s b) -> p s b", s=2
                    ),
                )

            # z order i, g, f, o: i*g computes mid-iteration, f*c + tanh(c')
            # during o's matmuls, so the post-last-psum chain is just o's
            # activation, h = o*th, and the store.
            SEQ = (0, 3, 1, 2)
            FN = {0: SIG, 1: SIG, 2: SIG, 3: TANH}
            for ch in range(NCH):
                cs = slice(ch * 512, (ch + 1) * 512)
                for s in range(NS):
                    if ch == 0 and s < NS - 2:
                        load_wgrp(s + 2)
                    # gate z -> gate-tile id 8*z + s (weights in gorder tiles)
                    ps = {}
                    gt = {}
                    hc = outp.tile([128, 1024], bf16, name=f"hc{ch}_{s}", tag="hc")
                    cn = hc[:, 0:512]
                    hn = hc[:, 512:1024]
                    tmp = actp.tile([128, 512], bf16, name=f"tmp{ch}_{s}", tag="tmp")
                    th = actp.tile([128, 512], bf16, name=f"th{ch}_{s}", tag="th")
                    def mm_bf16(z, p):
                        g = 8 * z + s
                        if z == 1:
                            # boundary influence seeds the f-gate accumulator
                            nc.tensor.matmul(
                                p, wb_t[:, s * 128 : (s + 1) * 128], bd_t[:, cs],
                                start=True, stop=False,
                            )
                        for k in range(KBF):
                            nc.tensor.matmul(
                                p, w_bf[g][:, k, :], at_bf_t[:, k, cs],
                                start=(k == 0 and z != 1),
                                stop=(NFP8 == 0 and k == KBF - 1),
                            )

                    def mm_f8(z, p):
                        g = 8 * z + s
                        for kp in range(0, NFP8, 2):
                            nc.tensor.matmul(
                                p,
                                w_f8[g][:, kp : kp + 2, :],
                                at_f8_t[:, kp : kp + 2, cs],
                                start=False,
                                stop=(kp == NFP8 - 2),
                                perf_mode=DR,
                            )

                    if ch == 0 and s == 0 and NFP8:
                        # First iteration: run every gate's bf16 phase up front
                        # so the PE has work while the fp8 activations stream.
                        for z in SEQ:
                            ps[z] = psp.tile(
                                [128, 512], f32, name=f"ps{ch}_{s}_{z}", tag="ps"
                            )
                            mm_bf16(z, ps[z])
                    for zi, z in enumerate(SEQ):
                        if not (ch == 0 and s == 0 and NFP8):
                            p = psp.tile(
                                [128, 512], f32, name=f"ps{ch}_{s}_{z}", tag="ps"
                            )
                            ps[z] = p
                            mm_bf16(z, p)
                            mm_f8(z, p)
                        else:
                            mm_f8(z, ps[z])
                        g = 8 * z + s
                        t = actp.tile(
                            [128, 512], bf16, name=f"g{ch}_{s}_{z}", tag=f"g{z}"
                        )
                        nc.scalar.activation(
                            t, ps[z], FN[z], bias=bias_t[:, g : g + 1],
                            scale=1.0 / SCALE,
                        )
                        gt[z] = t
                        # interleave elementwise as operands become ready
                        if zi == 1:  # i, g done
                            nc.vector.tensor_mul(tmp, gt[0], gt[3])
                        elif zi == 2:  # f done
                            nc.vector.tensor_mul(cn, gt[1], c_t[:, s, cs])
                            nc.vector.tensor_add(cn, cn, tmp)
                            nc.scalar.activation(th, cn, TANH)
                    nc.vector.tensor_mul(hn, gt[2], th)

                    off = (ch * NS + s) * 1024
                    nc.scalar.dma_start(out=hc_o[:, off : off + 1024], in_=hc)
    nc.compile()
    return nc


def _get_program():
    global _PROG
    if _PROG is None:
        _PROG = _build_program()
    return _PROG


def _prep_inputs(inputs):
    """Host-side marshalling into exact SBUF layouts (see header)."""
    f = np.float32
    bf = ml_dtypes.bfloat16
    f8 = ml_dtypes.float8_e4m3
    x = np.asarray(inputs["x"], f)
    h_prev = np.asarray(inputs["h_prev"], f)
    c_prev = np.asarray(inputs["c_prev"], f)
    boundary = np.asarray(inputs["boundary"], f)

    gates = ["i", "f", "o", "g"]
    # M [1536, 4096]: rows = contraction (x then h), cols = [i|f|o|g] x H.
    M = np.empty((KTOT, 4 * H), f)
    bias_vec = np.empty(4 * H, f)
    for zi, z in enumerate(gates):
        W = np.asarray(inputs[f"W_{z}"], f)
        U = np.asarray(inputs[f"U_{z}"], f)
        cs = slice(zi * H, (zi + 1) * H)
        M[:IN, cs] = W.T
        M[IN:, cs] = U.T
        b = np.asarray(inputs[f"b_W{z}"], f) + np.asarray(inputs[f"b_U{z}"], f)
        if z == "f":
            b = b + np.asarray(inputs["b_Wb"], f)
        bias_vec[cs] = b
    M *= SCALE

    # [K, 4H] -> [128, NG, KS, 128]: K-row = 128*kk + p, col = 128*g + c,
    # with gate-tiles permuted into compute order (s-major, then gate)
    gorder = [s + 8 * z for s in range(NS) for z in range(4)]

    def dev_weights(Msub, ks):
        t = Msub.reshape(ks, 128, NG, 128).transpose(1, 2, 0, 3)[:, gorder]
        return np.ascontiguousarray(t).reshape(128, -1)

    m_bf = dev_weights(M[: KBF * 128], KBF).astype(bf)
    bias_dev = np.ascontiguousarray(bias_vec.reshape(NG, 128).T)  # [128, NG]
    wb_dev = np.ascontiguousarray(
        (np.asarray(inputs["W_b"], f).T * SCALE).astype(bf)
    )  # [2, H]

    AT = np.concatenate([x, h_prev], axis=1).T  # [1536, 8192] (full batch)
    at_bf_full = np.ascontiguousarray(
        AT[: KBF * 128].reshape(KBF, 128, B).transpose(1, 0, 2)
    ).astype(bf)  # [128, KBF, B]
    cT = c_prev.T  # [H, B]

    in_maps = []
    if NFP8:
        m_f8 = dev_weights(M[KBF * 128 :], NFP8).astype(f8)
        at_f8_full = np.ascontiguousarray(
            AT[KBF * 128 :].reshape(NFP8, 128, B).transpose(1, 0, 2)
        ).astype(f8)  # [128, NFP8, B]
    for c in range(NCORES):
        rs = slice(c * BLOC, (c + 1) * BLOC)
        im = {
            "m_bf": m_bf,
            "at_bf": np.ascontiguousarray(at_bf_full[:, :, rs]).reshape(128, -1),
            "bias_in": bias_dev,
            "bd_in": np.ascontiguousarray(boundary[rs].T.astype(bf)),
            "wb_in": wb_dev,
            "c_in": np.ascontiguousarray(
                cT[:, rs].reshape(NS, 128, BLOC).transpose(1, 0, 2)
            ).reshape(128, -1).astype(bf),
        }
        if NFP8:
            im["m_f8"] = m_f8
            im["at_f8"] = np.ascontiguousarray(at_f8_full[:, :, rs]).reshape(128, -1)
        in_maps.append(im)
    return in_maps


def _unshard(res_list):
    """hc_out [128, NCH*NS*1024] bf16 per core -> (h, c) [B, H] f32 full."""
    h_parts, c_parts = [], []
    for r in res_list:
        t = np.asarray(r["hc_out"], dtype=np.float32)
        # [128, ch, s, half(c|h), 512] -> [h=128*s, b=512*ch+...]
        t = t.reshape(128, NCH, NS, 2, 512)
        core_c = t[:, :, :, 0, :].transpose(2, 0, 1, 3).reshape(H, BLOC)
        core_h = t[:, :, :, 1, :].transpose(2, 0, 1, 3).reshape(H, BLOC)
        c_parts.append(core_c.T)
        h_parts.append(core_h.T)
    return (
        np.ascontiguousarray(np.concatenate(h_parts, axis=0)),
        np.ascontiguousarray(np.concatenate(c_parts, axis=0)),
    )


def run(inputs, trace=False):
    """Returns ((h, c), BassKernelResults)."""
    from concourse.bass_utils import run_bass_kernel_spmd

    nc = _get_program()
    in_maps = _prep_inputs(inputs)
    res = run_bass_kernel_spmd(
        nc, in_maps, core_ids=list(range(NCORES)), trace=trace
    )
    h, c = _unshard(res.results)
    return (h, c), res


def kernel(**inputs):
    out, _ = run(inputs, trace=False)
    return out


# revision 26
# speedup vs baseline: 1.1690x; 1.0095x over previous
"""Trainium2 Bass kernel for a custom LSTM cell.

Math (per reference):
    i = sigmoid(x @ W_i.T + b_Wi + h @ U_i.T + b_Ui)
    f = sigmoid(x @ W_f.T + b_Wf + h @ U_f.T + b_Uf + boundary @ W_b.T + b_Wb)
    o = sigmoid(x @ W_o.T + b_Wo + h @ U_o.T + b_Uo)
    g = tanh   (x @ W_g.T + b_Wg + h @ U_g.T + b_Ug)
    c = f * c_prev + i * g
    h = o * tanh(c)

Strategy: data-parallel over batch across 8 NeuronCores (1024 rows each).
Unlike the batch-on-partitions baseline, PSUM output tiles put GATE dims on
partitions and batch on the free axis (out = M_tile.T @ A_tile).  That lets
the per-gate bias ride the scalar-engine activation instruction (per-partition
bias + scale), removing all dedicated bias matmuls from the PE.

Operands are bf16 (same PE rate as f32r, half the LDWEIGHTS and DMA cost).
The last NFP8 of the 8 h-contraction subtiles run as fp8e4m3 DoubleRow
matmuls (2 K-subtiles per instruction, 2x PE throughput).  All matmul
operands on the weight side are pre-scaled by 128 on the host so the fp8
U-weights sit in e4m3's normal range; the activation instruction's
scale=1/128 undoes it before sigmoid/tanh.

Host marshalling pre-arranges every tensor into the exact SBUF layout
(partition-major), so all DMAs are contiguous per partition.
"""

import sys

sys.path.insert(0, "/opt/trn_rl_repo")

import numpy as np
import ml_dtypes

B, IN, H = 8192, 512, 1024
NCORES = 8
BLOC = B // NCORES  # 1024 batch rows per core
KTOT = IN + H  # 1536 contraction
KT = KTOT // 128  # 12 K-subtiles total
NFP8 = 8  # of the 8 h-side K-subtiles, how many run fp8 DoubleRow (even)
KBF = KT - NFP8  # leading bf16 K-subtiles (x part + first h subtiles)
NG = 4 * H // 128  # 32 gate-tiles of 128 output dims
NS = H // 128  # 8 h-slices
NCH = BLOC // 512  # 2 batch chunks of 512
SCALE = 128.0  # weight pre-scale (power of two; undone in activation)

_PROG = None  # cached so repeat calls skip rebuild/recompile


def _build_program():
    import concourse.bass as bass
    import concourse.mybir as mybir
    import concourse.tile as tile
    from concourse import bacc
    from contextlib import ExitStack

    f32 = mybir.dt.float32
    bf16 = mybir.dt.bfloat16
    f8 = mybir.dt.float8e4
    SIG = mybir.ActivationFunctionType.Sigmoid
    TANH = mybir.ActivationFunctionType.Tanh
    DR = mybir.MatmulPerfMode.DoubleRow

    nc = bacc.Bacc("TRN2", target_bir_lowering=False, debug=False)

    # Device tensors, all in final SBUF layout (partition dim first).
    # m_bf: [128, NG, KBF, 128] bf16   weights, K-subtiles 0..KBF-1
    # m_f8: [128, NG, NFP8, 128] fp8   weights, K-subtiles KBF..11
    # at_bf: [128, KBF, BLOC] bf16     activations [x | h_lo]
    # at_f8: [128, NFP8, BLOC] fp8     activations h_hi
    # bias: [128, NG] f32              per gate-tile per partition
    # bd:   [2, BLOC] bf16             boundary.T
    # wb:   [2, H] bf16                W_b.T * SCALE
    # c_in: [128, NS, BLOC] f32        c_prev.T
    # h/c out: [128, NS, BLOC] f32     transposed outputs
    m_bf_d = nc.dram_tensor("m_bf", [128, NG * KBF * 128], bf16, kind="ExternalInput").ap()
    at_bf_d = nc.dram_tensor("at_bf", [128, KBF * BLOC], bf16, kind="ExternalInput").ap()
    bias_d = nc.dram_tensor("bias_in", [128, NG], f32, kind="ExternalInput").ap()
    bd_d = nc.dram_tensor("bd_in", [2, BLOC], bf16, kind="ExternalInput").ap()
    wb_d = nc.dram_tensor("wb_in", [2, H], bf16, kind="ExternalInput").ap()
    c_d = nc.dram_tensor("c_in", [128, NS * BLOC], bf16, kind="ExternalInput").ap()
    # merged output: slot (ch, s) holds [c' | h'] as [128, 1024]
    hc_o = nc.dram_tensor(
        "hc_out", [128, NCH * NS * 1024], bf16, kind="ExternalOutput"
    ).ap()
    if NFP8:
        m_f8_d = nc.dram_tensor(
            "m_f8", [128, NG * NFP8 * 128], f8, kind="ExternalInput"
        ).ap()
        at_f8_d = nc.dram_tensor(
            "at_f8", [128, NFP8 * BLOC], f8, kind="ExternalInput"
        ).ap()

    with tile.TileContext(nc) as tc:
        with ExitStack() as ctx:
            wup = ctx.enter_context(tc.tile_pool(name="wup", bufs=1))
            cst = ctx.enter_context(tc.tile_pool(name="cst", bufs=1))
            wp = ctx.enter_context(tc.tile_pool(name="wp", bufs=1))
            actp = ctx.enter_context(tc.tile_pool(name="actp", bufs=2))
            outp = ctx.enter_context(tc.tile_pool(name="outp", bufs=4))
            psp = ctx.enter_context(tc.tile_pool(name="psp", bufs=8, space="PSUM"))

            # PE warm-up: dummy bf16 matmuls with no DMA deps push the PE
            # through its p-state ramp while the first weights load.
            wu_w = wup.tile([128, 128], bf16, name="wu_w")
            nc.vector.memset(wu_w, 0.0)
            wu_ps = psp.tile([128, 512], f32, name="wu_ps", tag="ps")
            for _ in range(48):
                nc.tensor.matmul(wu_ps[:, 0:128], wu_w, wu_w, start=True, stop=True)

            # Queue plan (DMA engines are shared; sem pool is small, so few,
            # fat DMAs, with the first-needed data leading each queue):
            #  - sync/Q1:  activations, one DMA per K-subtile (PE start gate)
            #  - scalar/Q10: tiny constants, then weights as one fat DMA per
            #    4-gate-tile group in compute order (big per-partition runs
            #    -> big packets -> ~250GB/s)
            #  - gpsimd/Q0: c_prev (bf16, 4 DMAs), then output stores
            at_bf_t = cst.tile([128, KBF, BLOC], bf16, name="at_bf_t")
            at_f8_t = cst.tile([128, NFP8, BLOC], f8, name="at_f8_t") if NFP8 else None
            for k in range(KBF):
                nc.sync.dma_start(
                    out=at_bf_t[:, k], in_=at_bf_d[:, k * BLOC : (k + 1) * BLOC]
                )
            for kp in range(0, NFP8, 2):
                nc.gpsimd.dma_start(
                    out=at_f8_t[:, kp : kp + 2],
                    in_=at_f8_d[:, kp * BLOC : (kp + 2) * BLOC].rearrange(
                        "p (k b) -> p k b", k=2
                    ),
                )

            bias_t = cst.tile([128, NG], f32, name="bias_t")
            nc.scalar.dma_start(out=bias_t, in_=bias_d[:, :])
            bd_t = cst.tile([2, BLOC], bf16, name="bd_t")
            nc.scalar.dma_start(out=bd_t, in_=bd_d[:, :])
            wb_t = cst.tile([2, H], bf16, name="wb_t")
            nc.scalar.dma_start(out=wb_t, in_=wb_d[:, :])

            # weights: host already permuted gate-tiles into compute order
            # (gorder); weight group j = the 4 gates of h-slice j.  Groups 0-1
            # load up front; later groups are issued from inside the compute
            # loop so the in-order scalar queue paces them behind compute,
            # keeping early DMA bandwidth for the activations.
            gorder = [s + 8 * z for s in range(NS) for z in range(4)]
            w_bf = {}
            w_f8 = {}
            wgrp_tiles = []
            for gi in range(0, NG, 4):
                t = wp.tile([128, 4, KBF, 128], bf16, name=f"wbf_g{gi}")
                t8 = (
                    wp.tile([128, 4, NFP8, 128], f8, name=f"wf8_g{gi}")
                    if NFP8
                    else None
                )
                wgrp_tiles.append((t, t8))
                for j in range(4):
                    g = gorder[gi + j]
                    w_bf[g] = t[:, j]
                    if NFP8:
                        w_f8[g] = t8[:, j]

            def load_wgrp(grp, split=False):
                gi = grp * 4
                t, t8 = wgrp_tiles[grp]
                if split:  # per-gate-tile DMAs so the first tile lands early
                    for j in range(4):
                        nc.scalar.dma_start(
                            out=t[:, j],
                            in_=m_bf_d[
                                :, (gi + j) * KBF * 128 : (gi + j + 1) * KBF * 128
                            ].rearrange("p (k c) -> p k c", k=KBF),
                        )
                        if NFP8:
                            nc.scalar.dma_start(
                                out=t8[:, j],
                                in_=m_f8_d[
                                    :,
                                    (gi + j) * NFP8 * 128 : (gi + j + 1) * NFP8 * 128,
                                ].rearrange("p (k c) -> p k c", k=NFP8),
                            )
                    return
                nc.scalar.dma_start(
                    out=t,
                    in_=m_bf_d[
                        :, gi * KBF * 128 : (gi + 4) * KBF * 128
                    ].rearrange("p (j k c) -> p j k c", j=4, k=KBF),
                )
                if NFP8:
                    nc.scalar.dma_start(
                        out=t8,
                        in_=m_f8_d[
                            :, gi * NFP8 * 128 : (gi + 4) * NFP8 * 128
                        ].rearrange("p (j k c) -> p j k c", j=4, k=NFP8),
                    )

            c_t = cst.tile([128, NS, BLOC], bf16, name="c_t")

            def load_c_pair(sp):
                nc.scalar.dma_start(
                    out=c_t[:, sp : sp + 2],
                    in_=c_d[:, sp * BLOC : (sp + 2) * BLOC].rearrange(
                        "p (s b) -> p s b", s=2
                    ),
                )

            load_wgrp(0, split=True)
            load_c_pair(0)
            load_wgrp(1)
            load_c_pair(2)

            # z order i, g, f, o: i*g computes mid-iteration, f*c + tanh(c')
            # during o's matmuls, so the post-last-psum chain is just o's
            # activation, h = o*th, and the store.
            SEQ = (0, 3, 1, 2)
            FN = {0: SIG, 1: SIG, 2: SIG, 3: TANH}
            for ch in range(NCH):
                cs = slice(ch * 512, (ch + 1) * 512)
                for s in range(NS):
                    if ch == 0 and s < NS - 2:
                        load_wgrp(s + 2)
                    if ch == 0 and s in (1, 3):
                        load_c_pair(s + 3)  # pairs (4,5) and (6,7)
                    # gate z -> gate-tile id 8*z + s (weights in gorder tiles)
                    ps = {}
                    gt = {}
                    hc = outp.tile([128, 1024], bf16, name=f"hc{ch}_{s}", tag="hc")
                    cn = hc[:, 0:512]
                    hn = hc[:, 512:1024]
                    tmp = actp.tile([128, 512], bf16, name=f"tmp{ch}_{s}", tag="tmp")
                    th = actp.tile([128, 512], bf16, name=f"th{ch}_{s}", tag="th")
                    def mm_bf16(z, p):
                        g = 8 * z + s
                        if z == 1:
                            # boundary influence seeds the f-gate accumulator
                            nc.tensor.matmul(
                                p, wb_t[:, s * 128 : (s + 1) * 128], bd_t[:, cs],
                                start=True, stop=False,
                            )
                        for k in range(KBF):
                            nc.tensor.matmul(
                                p, w_bf[g][:, k, :], at_bf_t[:, k, cs],
                                start=(k == 0 and z != 1),
                                stop=(NFP8 == 0 and k == KBF - 1),
                            )

                    def mm_f8(z, p):
                        g = 8 * z + s
                        for kp in range(0, NFP8, 2):
                            nc.tensor.matmul(
                                p,
                                w_f8[g][:, kp : kp + 2, :],
                                at_f8_t[:, kp : kp + 2, cs],
                                start=False,
                                stop=(kp == NFP8 - 2),
                                perf_mode=DR,
                            )

                    if ch == 0 and s == 0 and NFP8:
                        # First iteration: run every gate's bf16 phase up front
                        # so the PE has work while the fp8 activations stream.
                        for z in SEQ:
                            ps[z] = psp.tile(
                                [128, 512], f32, name=f"ps{ch}_{s}_{z}", tag="ps"
                            )
                            mm_bf16(z, ps[z])
                    for zi, z in enumerate(SEQ):
                        if not (ch == 0 and s == 0 and NFP8):
                            p = psp.tile(
                                [128, 512], f32, name=f"ps{ch}_{s}_{z}", tag="ps"
                            )
                            ps[z] = p
                            mm_bf16(z, p)
                            mm_f8(z, p)
                        else:
                            mm_f8(z, ps[z])
                        g = 8 * z + s
                        t = actp.tile(
                            [128, 512], bf16, name=f"g{ch}_{s}_{z}", tag=f"g{z}"
                        )
                        nc.scalar.activation(
                            t, ps[z], FN[z], bias=bias_t[:, g : g + 1],
                            scale=1.0 / SCALE,
                        )
                        gt[z] = t
                        # interleave elementwise as operands become ready
                        if zi == 1:  # i, g done
                            nc.vector.tensor_mul(tmp, gt[0], gt[3])
                        elif zi == 2:  # f done
                            nc.vector.tensor_mul(cn, gt[1], c_t[:, s, cs])
                            nc.vector.tensor_add(cn, cn, tmp)
                            nc.scalar.activation(th, cn, TANH)
                    nc.vector.tensor_mul(hn, gt[2], th)

                    off = (ch * NS + s) * 1024
                    nc.scalar.dma_start(out=hc_o[:, off : off + 1024], in_=hc)
    nc.compile()
    return nc


def _get_program():
    global _PROG
    if _PROG is None:
        _PROG = _build_program()
    return _PROG


def _prep_inputs(inputs):
    """Host-side marshalling into exact SBUF layouts (see header)."""
    f = np.float32
    bf = ml_dtypes.bfloat16
    f8 = ml_dtypes.float8_e4m3
    x = np.asarray(inputs["x"], f)
    h_prev = np.asarray(inputs["h_prev"], f)
    c_prev = np.asarray(inputs["c_prev"], f)
    boundary = np.asarray(inputs["boundary"], f)

    gates = ["i", "f", "o", "g"]
    # M [1536, 4096]: rows = contraction (x then h), cols = [i|f|o|g] x H.
    M = np.empty((KTOT, 4 * H), f)
    bias_vec = np.empty(4 * H, f)
    for zi, z in enumerate(gates):
        W = np.asarray(inputs[f"W_{z}"], f)
        U = np.asarray(inputs[f"U_{z}"], f)
        cs = slice(zi * H, (zi + 1) * H)
        M[:IN, cs] = W.T
        M[IN:, cs] = U.T
        b = np.asarray(inputs[f"b_W{z}"], f) + np.asarray(inputs[f"b_U{z}"], f)
        if z == "f":
            b = b + np.asarray(inputs["b_Wb"], f)
        bias_vec[cs] = b
    M *= SCALE

    # [K, 4H] -> [128, NG, KS, 128]: K-row = 128*kk + p, col = 128*g + c,
    # with gate-tiles permuted into compute order (s-major, then gate)
    gorder = [s + 8 * z for s in range(NS) for z in range(4)]

    def dev_weights(Msub, ks):
        t = Msub.reshape(ks, 128, NG, 128).transpose(1, 2, 0, 3)[:, gorder]
        return np.ascontiguousarray(t).reshape(128, -1)

    m_bf = dev_weights(M[: KBF * 128], KBF).astype(bf)
    bias_dev = np.ascontiguousarray(bias_vec.reshape(NG, 128).T)  # [128, NG]
    wb_dev = np.ascontiguousarray(
        (np.asarray(inputs["W_b"], f).T * SCALE).astype(bf)
    )  # [2, H]

    AT = np.concatenate([x, h_prev], axis=1).T  # [1536, 8192] (full batch)
    at_bf_full = np.ascontiguousarray(
        AT[: KBF * 128].reshape(KBF, 128, B).transpose(1, 0, 2)
    ).astype(bf)  # [128, KBF, B]
    cT = c_prev.T  # [H, B]

    in_maps = []
    if NFP8:
        m_f8 = dev_weights(M[KBF * 128 :], NFP8).astype(f8)
        at_f8_full = np.ascontiguousarray(
            AT[KBF * 128 :].reshape(NFP8, 128, B).transpose(1, 0, 2)
        ).astype(f8)  # [128, NFP8, B]
    for c in range(NCORES):
        rs = slice(c * BLOC, (c + 1) * BLOC)
        im = {
            "m_bf": m_bf,
            "at_bf": np.ascontiguousarray(at_bf_full[:, :, rs]).reshape(128, -1),
            "bias_in": bias_dev,
            "bd_in": np.ascontiguousarray(boundary[rs].T.astype(bf)),
            "wb_in": wb_dev,
            "c_in": np.ascontiguousarray(
                cT[:, rs].reshape(NS, 128, BLOC).transpose(1, 0, 2)
            ).reshape(128, -1).astype(bf),
        }
        if NFP8:
            im["m_f8"] = m_f8
            im["at_f8"] = np.ascontiguousarray(at_f8_full[:, :, rs]).reshape(128, -1)
        in_maps.append(im)
    return in_maps


def _unshard(res_list):
    """hc_out [128, NCH*NS*1024] bf16 per core -> (h, c) [B, H] f32 full."""
    h_parts, c_parts = [], []
    for r in res_list:
        t = np.asarray(r["hc_out"], dtype=np.float32)
        # [128, ch, s, half(c|h), 512] -> [h=128*s, b=512*ch+...]
        t = t.reshape(128, NCH, NS, 2, 512)
        core_c = t[:, :, :, 0, :].transpose(2, 0, 1, 3).reshape(H, BLOC)
        core_h = t[:, :, :, 1, :].transpose(2, 0, 1, 3).reshape(H, BLOC)
        c_parts.append(core_c.T)
        h_parts.append(core_h.T)
    return (
        np.ascontiguousarray(np.concatenate(h_parts, axis=0)),
        np.ascontiguousarray(np.concatenate(c_parts, axis=0)),
    )


def run(inputs, trace=False):
    """Returns ((h, c), BassKernelResults)."""
    from concourse.bass_utils import run_bass_kernel_spmd

    nc = _get_program()
    in_maps = _prep_inputs(inputs)
    res = run_bass_kernel_spmd(
        nc, in_maps, core_ids=list(range(NCORES)), trace=trace
    )
    h, c = _unshard(res.results)
    return (h, c), res


def kernel(**inputs):
    out, _ = run(inputs, trace=False)
    return out


# revision 35
# speedup vs baseline: 1.1741x; 1.0043x over previous
"""Trainium2 Bass kernel for a custom LSTM cell.

Math (per reference):
    i = sigmoid(x @ W_i.T + b_Wi + h @ U_i.T + b_Ui)
    f = sigmoid(x @ W_f.T + b_Wf + h @ U_f.T + b_Uf + boundary @ W_b.T + b_Wb)
    o = sigmoid(x @ W_o.T + b_Wo + h @ U_o.T + b_Uo)
    g = tanh   (x @ W_g.T + b_Wg + h @ U_g.T + b_Ug)
    c = f * c_prev + i * g
    h = o * tanh(c)

Strategy: data-parallel over batch across 8 NeuronCores (1024 rows each).
Unlike the batch-on-partitions baseline, PSUM output tiles put GATE dims on
partitions and batch on the free axis (out = M_tile.T @ A_tile).  That lets
the per-gate bias ride the scalar-engine activation instruction (per-partition
bias + scale), removing all dedicated bias matmuls from the PE.

Operands are bf16 (same PE rate as f32r, half the LDWEIGHTS and DMA cost).
The last NFP8 of the 8 h-contraction subtiles run as fp8e4m3 DoubleRow
matmuls (2 K-subtiles per instruction, 2x PE throughput).  All matmul
operands on the weight side are pre-scaled by 128 on the host so the fp8
U-weights sit in e4m3's normal range; the activation instruction's
scale=1/128 undoes it before sigmoid/tanh.

Host marshalling pre-arranges every tensor into the exact SBUF layout
(partition-major), so all DMAs are contiguous per partition.
"""

import sys

sys.path.insert(0, "/opt/trn_rl_repo")

import numpy as np
import ml_dtypes

B, IN, H = 8192, 512, 1024
NCORES = 8
BLOC = B // NCORES  # 1024 batch rows per core
KTOT = IN + H  # 1536 contraction
KT = KTOT // 128  # 12 K-subtiles total
NFP8 = 8  # of the 8 h-side K-subtiles, how many run fp8 DoubleRow (even)
KBF = KT - NFP8  # leading bf16 K-subtiles (x part + first h subtiles)
NG = 4 * H // 128  # 32 gate-tiles of 128 output dims
NS = H // 128  # 8 h-slices
NCH = BLOC // 512  # 2 batch chunks of 512
SCALE = 128.0  # weight pre-scale (power of two; undone in activation)

_PROG = None  # cached so repeat calls skip rebuild/recompile


def _build_program():
    import concourse.bass as bass
    import concourse.mybir as mybir
    import concourse.tile as tile
    from concourse import bacc
    from contextlib import ExitStack

    f32 = mybir.dt.float32
    bf16 = mybir.dt.bfloat16
    f8 = mybir.dt.float8e4
    SIG = mybir.ActivationFunctionType.Sigmoid
    TANH = mybir.ActivationFunctionType.Tanh
    DR = mybir.MatmulPerfMode.DoubleRow

    nc = bacc.Bacc("TRN2", target_bir_lowering=False, debug=False)

    # Device tensors, all in final SBUF layout (partition dim first).
    # m_bf: [128, NG, KBF, 128] bf16   weights, K-subtiles 0..KBF-1
    # m_f8: [128, NG, NFP8, 128] fp8   weights, K-subtiles KBF..11
    # at_bf: [128, KBF, BLOC] bf16     activations [x | h_lo]
    # at_f8: [128, NFP8, BLOC] fp8     activations h_hi
    # bias: [128, NG] f32              per gate-tile per partition
    # bd:   [2, BLOC] bf16             boundary.T
    # wb:   [2, H] bf16                W_b.T * SCALE
    # c_in: [128, NS*BLOC] bf16        c_prev.T
    # hc_out: [128, NCH*NS*1024] bf16  merged [c'|h'] per (chunk, h-slice)
    m_bf_d = nc.dram_tensor("m_bf", [128, NG * KBF * 128], bf16, kind="ExternalInput").ap()
    at_bf_d = nc.dram_tensor("at_bf", [128, KBF * BLOC], bf16, kind="ExternalInput").ap()
    bias_d = nc.dram_tensor("bias_in", [128, NG], f32, kind="ExternalInput").ap()
    bd_d = nc.dram_tensor("bd_in", [2, BLOC], bf16, kind="ExternalInput").ap()
    wb_d = nc.dram_tensor("wb_in", [2, H], bf16, kind="ExternalInput").ap()
    c_d = nc.dram_tensor("c_in", [128, NS * BLOC], bf16, kind="ExternalInput").ap()
    # merged output: slot (ch, s) holds [c' | h'] as [128, 1024]
    hc_o = nc.dram_tensor(
        "hc_out", [128, NCH * NS * 1024], bf16, kind="ExternalOutput"
    ).ap()
    if NFP8:
        m_f8_d = nc.dram_tensor(
            "m_f8", [128, NG * NFP8 * 128], f8, kind="ExternalInput"
        ).ap()
        at_f8_d = nc.dram_tensor(
            "at_f8", [128, NFP8 * BLOC], f8, kind="ExternalInput"
        ).ap()

    with tile.TileContext(nc) as tc:
        with ExitStack() as ctx:
            wup = ctx.enter_context(tc.tile_pool(name="wup", bufs=1))
            cst = ctx.enter_context(tc.tile_pool(name="cst", bufs=1))
            wp = ctx.enter_context(tc.tile_pool(name="wp", bufs=1))
            actp = ctx.enter_context(tc.tile_pool(name="actp", bufs=2))
            outp = ctx.enter_context(tc.tile_pool(name="outp", bufs=4))
            psp = ctx.enter_context(tc.tile_pool(name="psp", bufs=8, space="PSUM"))

            # PE warm-up: dummy bf16 matmuls with no DMA deps push the PE
            # through its p-state ramp while the first weights load.
            wu_w = wup.tile([128, 128], bf16, name="wu_w")
            nc.vector.memset(wu_w, 0.0)
            wu_ps = psp.tile([128, 512], f32, name="wu_ps", tag="ps")
            for _ in range(48):
                nc.tensor.matmul(wu_ps[:, 0:128], wu_w, wu_w, start=True, stop=True)

            # Queue plan (DMA engines are shared; sem pool is small, so few,
            # fat DMAs, with the first-needed data leading each queue):
            #  - sync/Q1:  activations, one DMA per K-subtile (PE start gate)
            #  - scalar/Q10: tiny constants, then weights as one fat DMA per
            #    4-gate-tile group in compute order (big per-partition runs
            #    -> big packets -> ~250GB/s)
            #  - gpsimd/Q0: c_prev (bf16, 4 DMAs), then output stores
            at_bf_t = cst.tile([128, KBF, BLOC], bf16, name="at_bf_t")
            at_f8_t = cst.tile([128, NFP8, BLOC], f8, name="at_f8_t") if NFP8 else None
            at_bf_3d = at_bf_d[:, :].rearrange("p (k b) -> p k b", k=KBF)
            at_f8_3d = at_f8_d[:, :].rearrange("p (k b) -> p k b", k=NFP8)
            # chunk-0 halves of the fp8 activations + the group-0 fp8 weights
            # on the gpsimd queue (shares arbitration priority with sync)
            nc.gpsimd.dma_start(
                out=at_f8_t[:, :, 0:512], in_=at_f8_3d[:, :, 0:512]
            )

            bias_t = cst.tile([128, NG], f32, name="bias_t")
            nc.scalar.dma_start(out=bias_t, in_=bias_d[:, :])
            bd_t = cst.tile([2, BLOC], bf16, name="bd_t")
            nc.scalar.dma_start(out=bd_t, in_=bd_d[:, :])
            wb_t = cst.tile([2, H], bf16, name="wb_t")
            nc.scalar.dma_start(out=wb_t, in_=wb_d[:, :])

            # weights: host already permuted gate-tiles into compute order
            # (gorder); weight group j = the 4 gates of h-slice j.  Groups 0-1
            # load up front; later groups are issued from inside the compute
            # loop so the in-order scalar queue paces them behind compute,
            # keeping early DMA bandwidth for the activations.
            gorder = [s + 8 * z for s in range(NS) for z in range(4)]
            w_bf = {}
            w_f8 = {}
            wgrp_tiles = []
            for gi in range(0, NG, 4):
                t = wp.tile([128, 4, KBF, 128], bf16, name=f"wbf_g{gi}")
                t8 = (
                    wp.tile([128, 4, NFP8, 128], f8, name=f"wf8_g{gi}")
                    if NFP8
                    else None
                )
                wgrp_tiles.append((t, t8))
                for j in range(4):
                    g = gorder[gi + j]
                    w_bf[g] = t[:, j]
                    if NFP8:
                        w_f8[g] = t8[:, j]

            def load_wgrp(grp):
                gi = grp * 4
                t, t8 = wgrp_tiles[grp]
                nc.scalar.dma_start(
                    out=t,
                    in_=m_bf_d[
                        :, gi * KBF * 128 : (gi + 4) * KBF * 128
                    ].rearrange("p (j k c) -> p j k c", j=4, k=KBF),
                )
                if NFP8:
                    nc.scalar.dma_start(
                        out=t8,
                        in_=m_f8_d[
                            :, gi * NFP8 * 128 : (gi + 4) * NFP8 * 128
                        ].rearrange("p (j k c) -> p j k c", j=4, k=NFP8),
                    )

            c_t = cst.tile([128, NS, BLOC], bf16, name="c_t")

            def load_c_pair(sp):
                nc.scalar.dma_start(
                    out=c_t[:, sp : sp + 2],
                    in_=c_d[:, sp * BLOC : (sp + 2) * BLOC].rearrange(
                        "p (s b) -> p s b", s=2
                    ),
                )

            # Iteration-0-critical data interleaved on the sync queue (low
            # queue ids win DMA-engine arbitration): at chunk-0 subtile k,
            # then weight tile j=k of group 0, alternating.
            t0, t80 = wgrp_tiles[0]
            for j in range(KBF):
                nc.sync.dma_start(
                    out=at_bf_t[:, j, 0:512], in_=at_bf_3d[:, j, 0:512]
                )
                nc.sync.dma_start(
                    out=t0[:, j],
                    in_=m_bf_d[:, j * KBF * 128 : (j + 1) * KBF * 128].rearrange(
                        "p (k c) -> p k c", k=KBF
                    ),
                )
            if NFP8:
                for j in range(4):
                    nc.gpsimd.dma_start(
                        out=t80[:, j],
                        in_=m_f8_d[
                            :, j * NFP8 * 128 : (j + 1) * NFP8 * 128
                        ].rearrange("p (k c) -> p k c", k=NFP8),
                    )
            load_c_pair(0)
            load_wgrp(1)
            load_c_pair(2)

            # z order i, g, f, o: i*g computes mid-iteration, f*c + tanh(c')
            # during o's matmuls, so the post-last-psum chain is just o's
            # activation, h = o*th, and the store.
            SEQ = (0, 3, 1, 2)
            FN = {0: SIG, 1: SIG, 2: SIG, 3: TANH}
            for ch in range(NCH):
                cs = slice(ch * 512, (ch + 1) * 512)
                for s in range(NS):
                    if ch == 0 and s < NS - 2:
                        load_wgrp(s + 2)
                    if ch == 0 and s in (1, 3):
                        load_c_pair(s + 3)  # pairs (4,5) and (6,7)
                    if ch == 0 and s == 2:
                        # chunk-1 halves of the activations, needed from ch=1
                        nc.scalar.dma_start(
                            out=at_bf_t[:, :, 512:1024],
                            in_=at_bf_3d[:, :, 512:1024],
                        )
                        if NFP8:
                            nc.scalar.dma_start(
                                out=at_f8_t[:, :, 512:1024],
                                in_=at_f8_3d[:, :, 512:1024],
                            )
                    # gate z -> gate-tile id 8*z + s (weights in gorder tiles)
                    ps = {}
                    gt = {}
                    hc = outp.tile([128, 1024], bf16, name=f"hc{ch}_{s}", tag="hc")
                    cn = hc[:, 0:512]
                    hn = hc[:, 512:1024]
                    tmp = actp.tile([128, 512], bf16, name=f"tmp{ch}_{s}", tag="tmp")
                    th = actp.tile([128, 512], bf16, name=f"th{ch}_{s}", tag="th")
                    def mm_bf16(z, p):
                        g = 8 * z + s
                        if z == 1:
                            # boundary influence seeds the f-gate accumulator
                            nc.tensor.matmul(
                                p, wb_t[:, s * 128 : (s + 1) * 128], bd_t[:, cs],
                                start=True, stop=False,
                            )
                        for k in range(KBF):
                            nc.tensor.matmul(
                                p, w_bf[g][:, k, :], at_bf_t[:, k, cs],
                                start=(k == 0 and z != 1),
                                stop=(NFP8 == 0 and k == KBF - 1),
                            )

                    def mm_f8(z, p):
                        g = 8 * z + s
                        for kp in range(0, NFP8, 2):
                            nc.tensor.matmul(
                                p,
                                w_f8[g][:, kp : kp + 2, :],
                                at_f8_t[:, kp : kp + 2, cs],
                                start=False,
                                stop=(kp == NFP8 - 2),
                                perf_mode=DR,
                            )

                    if ch == 0 and s == 0 and NFP8:
                        # First iteration: run every gate's bf16 phase up front
                        # so the PE has work while the fp8 activations stream.
                        for z in SEQ:
                            ps[z] = psp.tile(
                                [128, 512], f32, name=f"ps{ch}_{s}_{z}", tag="ps"
                            )
                            mm_bf16(z, ps[z])
                    for zi, z in enumerate(SEQ):
                        if not (ch == 0 and s == 0 and NFP8):
                            p = psp.tile(
                                [128, 512], f32, name=f"ps{ch}_{s}_{z}", tag="ps"
                            )
                            ps[z] = p
                            mm_bf16(z, p)
                            mm_f8(z, p)
                        else:
                            mm_f8(z, ps[z])
                        g = 8 * z + s
                        t = actp.tile(
                            [128, 512], bf16, name=f"g{ch}_{s}_{z}", tag=f"g{z}"
                        )
                        nc.scalar.activation(
                            t, ps[z], FN[z], bias=bias_t[:, g : g + 1],
                            scale=1.0 / SCALE,
                        )
                        gt[z] = t
                        # interleave elementwise as operands become ready
                        if zi == 1:  # i, g done
                            nc.vector.tensor_mul(tmp, gt[0], gt[3])
                        elif zi == 2:  # f done
                            nc.vector.tensor_mul(cn, gt[1], c_t[:, s, cs])
                            nc.vector.tensor_add(cn, cn, tmp)
                            nc.scalar.activation(th, cn, TANH)
                    nc.vector.tensor_mul(hn, gt[2], th)

                    off = (ch * NS + s) * 1024
                    nc.scalar.dma_start(out=hc_o[:, off : off + 1024], in_=hc)
    nc.compile()
    return nc


def _get_program():
    global _PROG
    if _PROG is None:
        _PROG = _build_program()
    return _PROG


def _prep_inputs(inputs):
    """Host-side marshalling into exact SBUF layouts (see header)."""
    f = np.float32
    bf = ml_dtypes.bfloat16
    f8 = ml_dtypes.float8_e4m3
    x = np.asarray(inputs["x"], f)
    h_prev = np.asarray(inputs["h_prev"], f)
    c_prev = np.asarray(inputs["c_prev"], f)
    boundary = np.asarray(inputs["boundary"], f)

    gates = ["i", "f", "o", "g"]
    # M [1536, 4096]: rows = contraction (x then h), cols = [i|f|o|g] x H.
    M = np.empty((KTOT, 4 * H), f)
    bias_vec = np.empty(4 * H, f)
    for zi, z in enumerate(gates):
        W = np.asarray(inputs[f"W_{z}"], f)
        U = np.asarray(inputs[f"U_{z}"], f)
        cs = slice(zi * H, (zi + 1) * H)
        M[:IN, cs] = W.T
        M[IN:, cs] = U.T
        b = np.asarray(inputs[f"b_W{z}"], f) + np.asarray(inputs[f"b_U{z}"], f)
        if z == "f":
            b = b + np.asarray(inputs["b_Wb"], f)
        bias_vec[cs] = b
    M *= SCALE

    # [K, 4H] -> [128, NG, KS, 128]: K-row = 128*kk + p, col = 128*g + c,
    # with gate-tiles permuted into compute order (s-major, then gate)
    gorder = [s + 8 * z for s in range(NS) for z in range(4)]

    def dev_weights(Msub, ks):
        t = Msub.reshape(ks, 128, NG, 128).transpose(1, 2, 0, 3)[:, gorder]
        return np.ascontiguousarray(t).reshape(128, -1)

    m_bf = dev_weights(M[: KBF * 128], KBF).astype(bf)
    bias_dev = np.ascontiguousarray(bias_vec.reshape(NG, 128).T)  # [128, NG]
    wb_dev = np.ascontiguousarray(
        (np.asarray(inputs["W_b"], f).T * SCALE).astype(bf)
    )  # [2, H]

    AT = np.concatenate([x, h_prev], axis=1).T  # [1536, 8192] (full batch)
    at_bf_full = np.ascontiguousarray(
        AT[: KBF * 128].reshape(KBF, 128, B).transpose(1, 0, 2)
    ).astype(bf)  # [128, KBF, B]
    cT = c_prev.T  # [H, B]

    in_maps = []
    if NFP8:
        m_f8 = dev_weights(M[KBF * 128 :], NFP8).astype(f8)
        at_f8_full = np.ascontiguousarray(
            AT[KBF * 128 :].reshape(NFP8, 128, B).transpose(1, 0, 2)
        ).astype(f8)  # [128, NFP8, B]
    for c in range(NCORES):
        rs = slice(c * BLOC, (c + 1) * BLOC)
        im = {
            "m_bf": m_bf,
            "at_bf": np.ascontiguousarray(at_bf_full[:, :, rs]).reshape(128, -1),
            "bias_in": bias_dev,
            "bd_in": np.ascontiguousarray(boundary[rs].T.astype(bf)),
            "wb_in": wb_dev,
            "c_in": np.ascontiguousarray(
                cT[:, rs].reshape(NS, 128, BLOC).transpose(1, 0, 2)
            ).reshape(128, -1).astype(bf),
        }
        if NFP8:
            im["m_f8"] = m_f8
            im["at_f8"] = np.ascontiguousarray(at_f8_full[:, :, rs]).reshape(128, -1)
        in_maps.append(im)
    return in_maps


def _unshard(res_list):
    """hc_out [128, NCH*NS*1024] bf16 per core -> (h, c) [B, H] f32 full."""
    h_parts, c_parts = [], []
    for r in res_list:
        t = np.asarray(r["hc_out"], dtype=np.float32)
        # [128, ch, s, half(c|h), 512] -> [h=128*s, b=512*ch+...]
        t = t.reshape(128, NCH, NS, 2, 512)
        core_c = t[:, :, :, 0, :].transpose(2, 0, 1, 3).reshape(H, BLOC)
        core_h = t[:, :, :, 1, :].transpose(2, 0, 1, 3).reshape(H, BLOC)
        c_parts.append(core_c.T)
        h_parts.append(core_h.T)
    return (
        np.ascontiguousarray(np.concatenate(h_parts, axis=0)),
        np.ascontiguousarray(np.concatenate(c_parts, axis=0)),
    )


def run(inputs, trace=False):
    """Returns ((h, c), BassKernelResults)."""
    from concourse.bass_utils import run_bass_kernel_spmd

    nc = _get_program()
    in_maps = _prep_inputs(inputs)
    res = run_bass_kernel_spmd(
        nc, in_maps, core_ids=list(range(NCORES)), trace=trace
    )
    h, c = _unshard(res.results)
    return (h, c), res


def kernel(**inputs):
    out, _ = run(inputs, trace=False)
    return out


# revision 44
# speedup vs baseline: 1.2076x; 1.0285x over previous
"""Trainium2 Bass kernel for a custom LSTM cell.

Math (per reference):
    i = sigmoid(x @ W_i.T + b_Wi + h @ U_i.T + b_Ui)
    f = sigmoid(x @ W_f.T + b_Wf + h @ U_f.T + b_Uf + boundary @ W_b.T + b_Wb)
    o = sigmoid(x @ W_o.T + b_Wo + h @ U_o.T + b_Uo)
    g = tanh   (x @ W_g.T + b_Wg + h @ U_g.T + b_Ug)
    c = f * c_prev + i * g
    h = o * tanh(c)

Strategy: data-parallel over batch across 8 NeuronCores (1024 rows each).
Unlike the batch-on-partitions baseline, PSUM output tiles put GATE dims on
partitions and batch on the free axis (out = M_tile.T @ A_tile).  That lets
the per-gate bias ride the scalar-engine activation instruction (per-partition
bias + scale), removing all dedicated bias matmuls from the PE.

Operands are bf16 (same PE rate as f32r, half the LDWEIGHTS and DMA cost).
The last NFP8 of the 8 h-contraction subtiles run as fp8e4m3 DoubleRow
matmuls (2 K-subtiles per instruction, 2x PE throughput).  All matmul
operands on the weight side are pre-scaled by 128 on the host so the fp8
U-weights sit in e4m3's normal range; the activation instruction's
scale=1/128 undoes it before sigmoid/tanh.

Host marshalling pre-arranges every tensor into the exact SBUF layout
(partition-major), so all DMAs are contiguous per partition.
"""

import sys

sys.path.insert(0, "/opt/trn_rl_repo")

import numpy as np
import ml_dtypes

B, IN, H = 8192, 512, 1024
NCORES = 8
BLOC = B // NCORES  # 1024 batch rows per core
KTOT = IN + H  # 1536 contraction
KT = KTOT // 128  # 12 K-subtiles total
NFP8 = 8  # of the 8 h-side K-subtiles, how many run fp8 DoubleRow (even)
KBF = KT - NFP8  # leading bf16 K-subtiles (x part + first h subtiles)
NG = 4 * H // 128  # 32 gate-tiles of 128 output dims
NS = H // 128  # 8 h-slices
NCH = BLOC // 512  # 2 batch chunks of 512
SCALE = 128.0  # weight pre-scale (power of two; undone in activation)

_PROG = None  # cached so repeat calls skip rebuild/recompile


def _build_program():
    import concourse.bass as bass
    import concourse.mybir as mybir
    import concourse.tile as tile
    from concourse import bacc
    from contextlib import ExitStack

    f32 = mybir.dt.float32
    bf16 = mybir.dt.bfloat16
    f8 = mybir.dt.float8e4
    SIG = mybir.ActivationFunctionType.Sigmoid
    TANH = mybir.ActivationFunctionType.Tanh
    DR = mybir.MatmulPerfMode.DoubleRow

    nc = bacc.Bacc("TRN2", target_bir_lowering=False, debug=False)

    # Device tensors, all in final SBUF layout (partition dim first).
    # m_bf: [128, NG, KBF, 128] bf16   weights, K-subtiles 0..KBF-1
    # m_f8: [128, NG, NFP8, 128] fp8   weights, K-subtiles KBF..11
    # at_bf: [128, KBF, BLOC] bf16     activations [x | h_lo]
    # at_f8: [128, NFP8, BLOC] fp8     activations h_hi
    # bias: [128, NG] f32              per gate-tile per partition
    # bd:   [2, BLOC] bf16             boundary.T
    # wb:   [2, H] bf16                W_b.T * SCALE
    # c_in: [128, NS*BLOC] bf16        c_prev.T
    # hc_out: [128, NCH*NS*1024] bf16  merged [c'|h'] per (chunk, h-slice)
    m_bf_d = nc.dram_tensor("m_bf", [128, NG * KBF * 128], bf16, kind="ExternalInput").ap()
    at_bf_d = nc.dram_tensor("at_bf", [128, KBF * BLOC], bf16, kind="ExternalInput").ap()
    bias_d = nc.dram_tensor("bias_in", [128, NG], f32, kind="ExternalInput").ap()
    # boundary rows broadcast across partitions + W_b columns per-partition
    bdb_d = nc.dram_tensor("bdb_in", [128, 2 * BLOC], bf16, kind="ExternalInput").ap()
    wbs_d = nc.dram_tensor("wbs_in", [128, NS * 2], f32, kind="ExternalInput").ap()
    c_d = nc.dram_tensor("c_in", [128, NS * BLOC], bf16, kind="ExternalInput").ap()
    # merged output: slot (ch, s) holds [c' | h'] as [128, 1024]
    hc_o = nc.dram_tensor(
        "hc_out", [128, NCH * NS * 1024], bf16, kind="ExternalOutput"
    ).ap()
    if NFP8:
        m_f8_d = nc.dram_tensor(
            "m_f8", [128, NG * NFP8 * 128], f8, kind="ExternalInput"
        ).ap()
        at_f8_d = nc.dram_tensor(
            "at_f8", [128, NFP8 * BLOC], f8, kind="ExternalInput"
        ).ap()

    with tile.TileContext(nc) as tc:
        with ExitStack() as ctx:
            wup = ctx.enter_context(tc.tile_pool(name="wup", bufs=1))
            cst = ctx.enter_context(tc.tile_pool(name="cst", bufs=1))
            wp = ctx.enter_context(tc.tile_pool(name="wp", bufs=1))
            actp = ctx.enter_context(tc.tile_pool(name="actp", bufs=2))
            outp = ctx.enter_context(tc.tile_pool(name="outp", bufs=4))
            psp = ctx.enter_context(tc.tile_pool(name="psp", bufs=8, space="PSUM"))

            # PE warm-up: dummy bf16 matmuls with no DMA deps push the PE
            # through its p-state ramp while the first weights load.
            wu_w = wup.tile([128, 128], bf16, name="wu_w")
            nc.vector.memset(wu_w, 0.0)
            wu_ps = psp.tile([128, 512], f32, name="wu_ps", tag="ps")
            for _ in range(48):
                nc.tensor.matmul(wu_ps[:, 0:128], wu_w, wu_w, start=True, stop=True)

            # Queue plan (DMA engines are shared; sem pool is small, so few,
            # fat DMAs, with the first-needed data leading each queue):
            #  - sync/Q1:  activations, one DMA per K-subtile (PE start gate)
            #  - scalar/Q10: tiny constants, then weights as one fat DMA per
            #    4-gate-tile group in compute order (big per-partition runs
            #    -> big packets -> ~250GB/s)
            #  - gpsimd/Q0: c_prev (bf16, 4 DMAs), then output stores
            at_bf_t = cst.tile([128, KBF, BLOC], bf16, name="at_bf_t")
            at_f8_t = cst.tile([128, NFP8, BLOC], f8, name="at_f8_t") if NFP8 else None
            at_bf_3d = at_bf_d[:, :].rearrange("p (k b) -> p k b", k=KBF)
            at_f8_3d = at_f8_d[:, :].rearrange("p (k b) -> p k b", k=NFP8)
            # chunk-0 halves of the fp8 activations + the group-0 fp8 weights
            # on the gpsimd queue (shares arbitration priority with sync)
            nc.gpsimd.dma_start(
                out=at_f8_t[:, :, 0:512], in_=at_f8_3d[:, :, 0:512]
            )

            bias_t = cst.tile([128, NG], f32, name="bias_t")
            nc.scalar.dma_start(out=bias_t, in_=bias_d[:, :])
            wbs_t = cst.tile([128, NS, 2], f32, name="wbs_t")
            nc.scalar.dma_start(
                out=wbs_t, in_=wbs_d[:, :].rearrange("p (s r) -> p s r", s=NS)
            )
            bdb_t = cst.tile([128, 2, BLOC], bf16, name="bdb_t")
            nc.scalar.dma_start(
                out=bdb_t[:, :, 0:512],
                in_=bdb_d[:, :].rearrange("p (r b) -> p r b", r=2)[:, :, 0:512],
            )

            # weights: host already permuted gate-tiles into compute order
            # (gorder); weight group j = the 4 gates of h-slice j.  Groups 0-1
            # load up front; later groups are issued from inside the compute
            # loop so the in-order scalar queue paces them behind compute,
            # keeping early DMA bandwidth for the activations.
            gorder = [s + 8 * z for s in range(NS) for z in range(4)]
            w_bf = {}
            w_f8 = {}
            wgrp_tiles = []
            for gi in range(0, NG, 4):
                t = wp.tile([128, 4, KBF, 128], bf16, name=f"wbf_g{gi}")
                t8 = (
                    wp.tile([128, 4, NFP8, 128], f8, name=f"wf8_g{gi}")
                    if NFP8
                    else None
                )
                wgrp_tiles.append((t, t8))
                for j in range(4):
                    g = gorder[gi + j]
                    w_bf[g] = t[:, j]
                    if NFP8:
                        w_f8[g] = t8[:, j]

            def load_wgrp(grp):
                gi = grp * 4
                t, t8 = wgrp_tiles[grp]
                nc.scalar.dma_start(
                    out=t,
                    in_=m_bf_d[
                        :, gi * KBF * 128 : (gi + 4) * KBF * 128
                    ].rearrange("p (j k c) -> p j k c", j=4, k=KBF),
                )
                if NFP8:
                    nc.scalar.dma_start(
                        out=t8,
                        in_=m_f8_d[
                            :, gi * NFP8 * 128 : (gi + 4) * NFP8 * 128
                        ].rearrange("p (j k c) -> p j k c", j=4, k=NFP8),
                    )

            c_t = cst.tile([128, NS, BLOC], bf16, name="c_t")

            def load_c_pair(sp):
                nc.scalar.dma_start(
                    out=c_t[:, sp : sp + 2],
                    in_=c_d[:, sp * BLOC : (sp + 2) * BLOC].rearrange(
                        "p (s b) -> p s b", s=2
                    ),
                )

            # Iteration-0-critical data interleaved on the sync queue (low
            # queue ids win DMA-engine arbitration): at chunk-0 subtile k,
            # then weight tile j=k of group 0, alternating.
            t0, t80 = wgrp_tiles[0]

            def load_w0j(j):
                nc.sync.dma_start(
                    out=t0[:, j],
                    in_=m_bf_d[:, j * KBF * 128 : (j + 1) * KBF * 128].rearrange(
                        "p (k c) -> p k c", k=KBF
                    ),
                )

            nc.sync.dma_start(out=at_bf_t[:, 0, 0:512], in_=at_bf_3d[:, 0, 0:512])
            load_w0j(0)
            for j in range(1, KBF):
                nc.sync.dma_start(
                    out=at_bf_t[:, j, 0:512], in_=at_bf_3d[:, j, 0:512]
                )
            # weight tiles in gate consumption order i(j0), g(j3), f(j1), o(j2)
            load_w0j(3)
            load_w0j(1)
            load_w0j(2)
            if NFP8:
                for j in (0, 3, 1, 2):  # gate consumption order i, g, f, o
                    nc.gpsimd.dma_start(
                        out=t80[:, j],
                        in_=m_f8_d[
                            :, j * NFP8 * 128 : (j + 1) * NFP8 * 128
                        ].rearrange("p (k c) -> p k c", k=NFP8),
                    )
            load_c_pair(0)
            load_wgrp(1)
            load_c_pair(2)

            # z order i, g, f, o: i*g computes mid-iteration, f*c + tanh(c')
            # during o's matmuls, so the post-last-psum chain is just o's
            # activation, h = o*th, and the store.
            SEQ = (0, 3, 1, 2)
            FN = {0: SIG, 1: SIG, 2: SIG, 3: TANH}
            for ch in range(NCH):
                cs = slice(ch * 512, (ch + 1) * 512)
                for s in range(NS):
                    if ch == 0 and s < NS - 2:
                        load_wgrp(s + 2)
                    if ch == 0 and s in (1, 3):
                        load_c_pair(s + 3)  # pairs (4,5) and (6,7)
                    if ch == 0 and s == 2:
                        # chunk-1 halves of the activations, needed from ch=1
                        nc.scalar.dma_start(
                            out=at_bf_t[:, :, 512:1024],
                            in_=at_bf_3d[:, :, 512:1024],
                        )
                        if NFP8:
                            nc.scalar.dma_start(
                                out=at_f8_t[:, :, 512:1024],
                                in_=at_f8_3d[:, :, 512:1024],
                            )
                        nc.scalar.dma_start(
                            out=bdb_t[:, :, 512:1024],
                            in_=bdb_d[:, :].rearrange("p (r b) -> p r b", r=2)[
                                :, :, 512:1024
                            ],
                        )
                    # gate z -> gate-tile id 8*z + s (weights in gorder tiles)
                    ps = {}
                    gt = {}
                    hc = outp.tile([128, 1024], bf16, name=f"hc{ch}_{s}", tag="hc")
                    cn = hc[:, 0:512]
                    hn = hc[:, 512:1024]
                    tmp = actp.tile([128, 512], bf16, name=f"tmp{ch}_{s}", tag="tmp")
                    th = actp.tile([128, 512], bf16, name=f"th{ch}_{s}", tag="th")
                    def mm_bf16(z, p):
                        g = 8 * z + s
                        if z == 1:
                            # boundary influence seeds the f-gate accumulator:
                            # DVE writes S*(bd @ W_b.T).T into the PSUM bank,
                            # then the matmuls accumulate on top (start=False)
                            b0 = actp.tile(
                                [128, 512], bf16, name=f"b0_{ch}_{s}", tag="b0"
                            )
                            b1 = actp.tile(
                                [128, 512], bf16, name=f"b1_{ch}_{s}", tag="b1"
                            )
                            nc.vector.tensor_scalar_mul(
                                b0, bdb_t[:, 0, cs], wbs_t[:, s, 0:1]
                            )
                            nc.vector.tensor_scalar_mul(
                                b1, bdb_t[:, 1, cs], wbs_t[:, s, 1:2]
                            )
                            nc.vector.tensor_add(p, b0, b1)
                        for k in range(KBF):
                            nc.tensor.matmul(
                                p, w_bf[g][:, k, :], at_bf_t[:, k, cs],
                                start=(k == 0 and z != 1),
                                stop=(NFP8 == 0 and k == KBF - 1),
                            )

                    def mm_f8(z, p):
                        g = 8 * z + s
                        for kp in range(0, NFP8, 2):
                            nc.tensor.matmul(
                                p,
                                w_f8[g][:, kp : kp + 2, :],
                                at_f8_t[:, kp : kp + 2, cs],
                                start=False,
                                stop=(kp == NFP8 - 2),
                                perf_mode=DR,
                            )

                    if ch == 0 and s == 0 and NFP8:
                        # First iteration: run every gate's bf16 phase up front
                        # so the PE has work while the fp8 activations stream.
                        for z in SEQ:
                            ps[z] = psp.tile(
                                [128, 512], f32, name=f"ps{ch}_{s}_{z}", tag="ps"
                            )
                            mm_bf16(z, ps[z])
                    for zi, z in enumerate(SEQ):
                        if not (ch == 0 and s == 0 and NFP8):
                            p = psp.tile(
                                [128, 512], f32, name=f"ps{ch}_{s}_{z}", tag="ps"
                            )
                            ps[z] = p
                            mm_bf16(z, p)
                            mm_f8(z, p)
                        else:
                            mm_f8(z, ps[z])
                        g = 8 * z + s
                        t = actp.tile(
                            [128, 512], bf16, name=f"g{ch}_{s}_{z}", tag=f"g{z}"
                        )
                        nc.scalar.activation(
                            t, ps[z], FN[z], bias=bias_t[:, g : g + 1],
                            scale=1.0 / SCALE,
                        )
                        gt[z] = t
                        # interleave elementwise as operands become ready
                        if zi == 1:  # i, g done
                            nc.vector.tensor_mul(tmp, gt[0], gt[3])
                        elif zi == 2:  # f done
                            nc.vector.tensor_mul(cn, gt[1], c_t[:, s, cs])
                            nc.vector.tensor_add(cn, cn, tmp)
                            nc.scalar.activation(th, cn, TANH)
                    nc.vector.tensor_mul(hn, gt[2], th)

                    off = (ch * NS + s) * 1024
                    nc.scalar.dma_start(out=hc_o[:, off : off + 1024], in_=hc)
    nc.compile()
    return nc


def _get_program():
    global _PROG
    if _PROG is None:
        _PROG = _build_program()
    return _PROG


def _prep_inputs(inputs):
    """Host-side marshalling into exact SBUF layouts (see header)."""
    f = np.float32
    bf = ml_dtypes.bfloat16
    f8 = ml_dtypes.float8_e4m3
    x = np.asarray(inputs["x"], f)
    h_prev = np.asarray(inputs["h_prev"], f)
    c_prev = np.asarray(inputs["c_prev"], f)
    boundary = np.asarray(inputs["boundary"], f)

    gates = ["i", "f", "o", "g"]
    # M [1536, 4096]: rows = contraction (x then h), cols = [i|f|o|g] x H.
    M = np.empty((KTOT, 4 * H), f)
    bias_vec = np.empty(4 * H, f)
    for zi, z in enumerate(gates):
        W = np.asarray(inputs[f"W_{z}"], f)
        U = np.asarray(inputs[f"U_{z}"], f)
        cs = slice(zi * H, (zi + 1) * H)
        M[:IN, cs] = W.T
        M[IN:, cs] = U.T
        b = np.asarray(inputs[f"b_W{z}"], f) + np.asarray(inputs[f"b_U{z}"], f)
        if z == "f":
            b = b + np.asarray(inputs["b_Wb"], f)
        bias_vec[cs] = b
    M *= SCALE

    # [K, 4H] -> [128, NG, KS, 128]: K-row = 128*kk + p, col = 128*g + c,
    # with gate-tiles permuted into compute order (s-major, then gate)
    gorder = [s + 8 * z for s in range(NS) for z in range(4)]

    def dev_weights(Msub, ks):
        t = Msub.reshape(ks, 128, NG, 128).transpose(1, 2, 0, 3)[:, gorder]
        return np.ascontiguousarray(t).reshape(128, -1)

    m_bf = dev_weights(M[: KBF * 128], KBF).astype(bf)
    bias_dev = np.ascontiguousarray(bias_vec.reshape(NG, 128).T)  # [128, NG]
    wb_dev = np.ascontiguousarray(
        (np.asarray(inputs["W_b"], f).T * SCALE).astype(bf)
    )  # [2, H]

    wbs_dev = np.ascontiguousarray(
        (np.asarray(inputs["W_b"], f) * SCALE).reshape(NS, 128, 2).transpose(1, 0, 2)
    ).reshape(128, -1)  # [128, NS*2]

    AT = np.concatenate([x, h_prev], axis=1).T  # [1536, 8192] (full batch)
    at_bf_full = np.ascontiguousarray(
        AT[: KBF * 128].reshape(KBF, 128, B).transpose(1, 0, 2)
    ).astype(bf)  # [128, KBF, B]
    cT = c_prev.T  # [H, B]

    in_maps = []
    if NFP8:
        m_f8 = dev_weights(M[KBF * 128 :], NFP8).astype(f8)
        at_f8_full = np.ascontiguousarray(
            AT[KBF * 128 :].reshape(NFP8, 128, B).transpose(1, 0, 2)
        ).astype(f8)  # [128, NFP8, B]
    for c in range(NCORES):
        rs = slice(c * BLOC, (c + 1) * BLOC)
        im = {
            "m_bf": m_bf,
            "at_bf": np.ascontiguousarray(at_bf_full[:, :, rs]).reshape(128, -1),
            "bias_in": bias_dev,
            "bdb_in": np.ascontiguousarray(
                np.broadcast_to(
                    boundary[rs].T.astype(bf).reshape(1, 2 * BLOC), (128, 2 * BLOC)
                )
            ),
            "wbs_in": wbs_dev,
            "c_in": np.ascontiguousarray(
                cT[:, rs].reshape(NS, 128, BLOC).transpose(1, 0, 2)
            ).reshape(128, -1).astype(bf),
        }
        if NFP8:
            im["m_f8"] = m_f8
            im["at_f8"] = np.ascontiguousarray(at_f8_full[:, :, rs]).reshape(128, -1)
        in_maps.append(im)
    return in_maps


def _unshard(res_list):
    """hc_out [128, NCH*NS*1024] bf16 per core -> (h, c) [B, H] f32 full."""
    h_parts, c_parts = [], []
    for r in res_list:
        t = np.asarray(r["hc_out"], dtype=np.float32)
        # [128, ch, s, half(c|h), 512] -> [h=128*s, b=512*ch+...]
        t = t.reshape(128, NCH, NS, 2, 512)
        core_c = t[:, :, :, 0, :].transpose(2, 0, 1, 3).reshape(H, BLOC)
        core_h = t[:, :, :, 1, :].transpose(2, 0, 1, 3).reshape(H, BLOC)
        c_parts.append(core_c.T)
        h_parts.append(core_h.T)
    return (
        np.ascontiguousarray(np.concatenate(h_parts, axis=0)),
        np.ascontiguousarray(np.concatenate(c_parts, axis=0)),
    )


def run(inputs, trace=False):
    """Returns ((h, c), BassKernelResults)."""
    from concourse.bass_utils import run_bass_kernel_spmd

    nc = _get_program()
    in_maps = _prep_inputs(inputs)
    res = run_bass_kernel_spmd(
        nc, in_maps, core_ids=list(range(NCORES)), trace=trace
    )
    h, c = _unshard(res.results)
    return (h, c), res


def kernel(**inputs):
    out, _ = run(inputs, trace=False)
    return out
